# revision 30
# baseline (speedup 1.0000x reference)
"""Trainium2 Bass kernel for nn_GATNet_IMG (dense 2-layer GAT, N=4096).

Sharding: 1D row-parallel over the node dim across 8 NeuronCores.
Each core computes Wh for its 512 rows (all 4 heads), AllGathers Wh
per head (overlapped with the next head's matmuls), then computes its
[512, 4096] attention block per head with a fused masked softmax (no
NxN matrix ever hits HBM), aggregates h^T = Wh^T @ u on TensorE, and
repeats the same pattern for the output attention layer.

Key tricks:
  - exp(leaky_relu(s)) == max(exp(s), exp(0.2*s))  (exact, all s)
  - attention logit halves f1/f2 come from one tiny [8]-wide matmul
    x @ B where B = [W[h]@a1[h] | W[h]@a2[h]] is host-precomputed, so
    logit precision is independent of the big-GEMM compute dtype
  - unnormalized attention + row-sum via ones-matmul; normalization is
    a post-scale of h^T (free-axis scale via a broadcast matmul)
  - elu(x) == max(exp(min(x, 0)) - 1, x)           (exact)
  - ln(2^-30) folded into the exp bias keeps row sums in DVE
    reciprocal range
  - host-side sharding pre-transposes x/adj AND interleaves all heavy
    streams into [128, ktiles, free] partition-major layouts so every
    DMA moves 8-64KB contiguous per partition (packet-rate limit)
"""

import math
from contextlib import ExitStack

import numpy as np

import concourse.bass as bass
import concourse.mybir as mybir
import concourse.tile as tile
from concourse import bass_utils
from concourse.masks import make_identity

F32 = mybir.dt.float32
F32R = mybir.dt.float32r
F16 = mybir.dt.float16
BF16 = mybir.dt.bfloat16
AF = mybir.ActivationFunctionType
OP = mybir.AluOpType

N = 4096
NFEAT = 4096
NHID = 1024
NHEADS = 4
BIT = 64
NC = 8
R = N // NC          # 512 attention rows per core
KT = NFEAT // 128    # 32 k tiles
JT = N // 128        # 32 node-column tiles
IT = R // 128        # 4 row tiles per core
ALPHA = 0.2
BIAS_LN = -30.0 * math.log(2.0)   # ln(2^-30) folded into exp bias (layer 1)


def _split_excess_waits(nc, max_waits=1):
    """walrus codegen rejects >max_waits sync-wait commands per instruction;
    push excess waits onto preceding same-engine NoOps."""
    n_fixed = 0
    for f in nc.m.functions:
        for b in f.blocks:
            new_insts = []
            changed = False
            for inst in b.instructions:
                si = getattr(inst, "sync_info", None)
                if si is not None and si.on_wait and len(si.on_wait) > max_waits:
                    waits = list(si.on_wait)
                    excess, keep = waits[:-max_waits], waits[-max_waits:]
                    for ci in range(0, len(excess), max_waits):
                        nop = mybir.InstNoOp(
                            name=f"{inst.name}-ws{ci}",
                            sync_info=mybir.SyncInfo(
                                on_wait=excess[ci:ci + max_waits], on_update=[]
                            ),
                            bass_nofuse=True,
                            engine=inst.engine,
                        )
                        new_insts.append(nop)
                    inst.sync_info = mybir.SyncInfo(
                        on_wait=keep, on_update=list(si.on_update or [])
                    )
                    n_fixed += 1
                    changed = True
                new_insts.append(inst)
            if changed:
                insts = b.instructions
                try:
                    b.instructions = new_insts
                except Exception:
                    while len(insts):
                        insts.pop()
                    for i in new_insts:
                        insts.append(i)
    return n_fixed


def build_program():
    nc = bass.Bass("TRN2", target_bir_lowering=False, debug=False, num_devices=NC)

    # host-interleaved inputs: [128 partitions, ktiles, free]
    x_d = nc.dram_tensor("x_sh", [128, KT, R], F32, kind="ExternalInput").ap()
    W_d = nc.dram_tensor("W_sh", [NHEADS, 128, KT, NHID], F16,
                         kind="ExternalInput").ap()
    B_d = nc.dram_tensor("B_sh", [128, KT, 2 * NHEADS], F32,
                         kind="ExternalInput").ap()
    adj_d = nc.dram_tensor("adj_sh", [128, JT, R], BF16, kind="ExternalInput").ap()
    wo_d = nc.dram_tensor("Wo_sh", [128, KT, BIT], F32, kind="ExternalInput").ap()
    a1o_d = nc.dram_tensor("a1_out", [BIT], F32, kind="ExternalInput").ap()
    a2o_d = nc.dram_tensor("a2_out", [BIT], F32, kind="ExternalInput").ap()
    out_d = nc.dram_tensor("out_rows", [R, BIT], F32, kind="ExternalOutput").ap()

    # collective bounce buffers; Wh per head so each head's allgather overlaps
    # the next head's phase-1 compute. Layout: [128, i*NHID] per rank so the
    # gathered output reads back with 8KB/partition lines.
    ag1_in = [nc.dram_tensor(f"ag1_in{h}", [128, IT * NHID], F16).ap()
              for h in range(NHEADS)]
    ag1_out = [nc.dram_tensor(f"ag1_out{h}", [NC * 128, IT * NHID], F16,
                              addr_space="Shared").ap() for h in range(NHEADS)]
    f_in = nc.dram_tensor("f_in", [1, NHEADS * R], F32).ap()
    f_out = nc.dram_tensor("f_out", [NC, NHEADS * R], F32, addr_space="Shared").ap()
    ag2_in = nc.dram_tensor("ag2_in", [128, IT * (BIT + 1)], F32).ap()
    ag2_out = nc.dram_tensor("ag2_out", [NC * 128, IT * (BIT + 1)], F32,
                             addr_space="Shared").ap()

    rg = [list(range(NC))]

    with tile.TileContext(nc) as tc, ExitStack() as ctx:
        cp = ctx.enter_context(tc.tile_pool(name="const", bufs=1))
        ident = cp.tile([128, 128], F32)
        make_identity(nc, ident)
        ones_col = cp.tile([128, 1], F32)
        nc.vector.memset(ones_col, 1.0)
        ones_row = cp.tile([1, 128], F32)
        nc.vector.memset(ones_row, 1.0)
        a1o_col = cp.tile([BIT, 1], F32)
        nc.sync.dma_start(a1o_col, a1o_d.rearrange("(b one) -> b one", one=1))
        a2o_b = cp.tile([128, BIT], F32)
        nc.sync.dma_start(
            a2o_b, a2o_d.rearrange("(one b) -> one b", one=1).to_broadcast([128, BIT]))
        # adjacency mask, resident for both attention layers (one big DMA)
        adjT = cp.tile([128, JT, R], BF16)
        nc.sync.dma_start(adjT, adj_d)
        # f1 broadcast tiles [128, R] per head
        f1b = [cp.tile([128, R], F32, name=f"f1b_{h}") for h in range(NHEADS)]

        # =============== phase 0: f-logit halves via x @ B ===============
        with tc.tile_pool(name="p0", bufs=1) as p0:
            with tc.tile_pool(name="p0ps", bufs=1, space="PSUM") as p0ps:
                bres = p0.tile([128, KT, 2 * NHEADS], F32R)
                nc.sync.dma_start(bres, B_d.bitcast(F32R))
                xres = p0.tile([128, KT, R], F32R)
                nc.sync.dma_start(xres, x_d.bitcast(F32R))
                bps = p0ps.tile([2 * NHEADS, R], F32)
                for k in range(KT):
                    nc.tensor.matmul(bps, lhsT=bres[:, k, :], rhs=xres[:, k, :],
                                     start=(k == 0), stop=(k == KT - 1))
                # f2 of my rows, all heads -> allgather (concat on partitions)
                fall = p0.tile([2 * NHEADS, R], F32)
                nc.vector.tensor_copy(fall, bps)
                nc.sync.dma_start(
                    f_in.rearrange("one (h j) -> (one h) j", h=NHEADS),
                    fall[NHEADS:, :])
                nc.gpsimd.collective_compute(
                    "AllGather", OP.bypass, ins=[f_in.opt()], outs=[f_out.opt()],
                    replica_groups=rg)
                for h in range(NHEADS):
                    # row h -> partition 0 (SBUF->SBUF DMA), then broadcast
                    f1row = p0.tile([1, R], F32, name=f"f1row{h}", tag="f1row")
                    nc.sync.dma_start(f1row, fall[h:h + 1, :])
                    fb_ps = p0ps.tile([128, R], F32, name=f"fb_ps{h}", tag="fbps")
                    nc.tensor.matmul(fb_ps, lhsT=ones_row, rhs=f1row,
                                     start=True, stop=True)
                    nc.vector.tensor_copy(f1b[h], fb_ps)

            # =============== phase 1: Wh = x @ W[h] ===============
            with tc.tile_pool(name="p1s", bufs=2) as p1s, \
                 tc.tile_pool(name="p1ps", bufs=1, space="PSUM") as p1ps, \
                 tc.tile_pool(name="p1d", bufs=2) as p1d:
                xp1 = p0.tile([128, KT, R], F16)
                nc.vector.tensor_copy(xp1, xres.bitcast(F32))
                for h in range(NHEADS):
                    ps = [[p1ps.tile([128, 512], F32, name=f"ps_{h}_{i}_{oh}",
                                     tag=f"ps{i}{oh}") for oh in range(2)]
                          for i in range(IT)]
                    for kb in range(4):
                        wres = p1s.tile([128, 8, NHID], F16, tag="wres")
                        nc.sync.dma_start(
                            wres, W_d[h, :, kb * 8:(kb + 1) * 8, :])
                        for kk in range(8):
                            k = kb * 8 + kk
                            for i in range(IT):
                                for oh in range(2):
                                    nc.tensor.matmul(
                                        ps[i][oh],
                                        lhsT=xp1[:, k, i * 128:(i + 1) * 128],
                                        rhs=wres[:, kk, oh * 512:(oh + 1) * 512],
                                        start=(k == 0), stop=(k == KT - 1),
                                    )
                    for i in range(IT):
                        wh_sb = p1d.tile([128, NHID], F16, tag="wh_sb")
                        nc.vector.tensor_copy(wh_sb[:, :512], ps[i][0])
                        nc.vector.tensor_copy(wh_sb[:, 512:], ps[i][1])
                        nc.sync.dma_start(
                            ag1_in[h][:, i * NHID:(i + 1) * NHID], wh_sb)
                    # allgather this head's Wh while later heads compute
                    nc.gpsimd.collective_compute(
                        "AllGather", OP.bypass, ins=[ag1_in[h].opt()],
                        outs=[ag1_out[h].opt()], replica_groups=rg)

        # =============== phase 2: attention + aggregate, per head ===============
        p2c = ctx.enter_context(tc.tile_pool(name="p2c", bufs=1))
        xcatT = p2c.tile([128, KT, R], F16)
        wof = p2c.tile([128, KT, BIT], F32)
        nc.sync.dma_start(wof, wo_d)
        wob = p2c.tile([128, KT, BIT], F16)
        nc.vector.tensor_copy(wob, wof)

        pps = ctx.enter_context(tc.tile_pool(name="pps", bufs=1, space="PSUM"))
        p2s = ctx.enter_context(tc.tile_pool(name="p2s", bufs=2))
        p2w = ctx.enter_context(tc.tile_pool(name="p2w", bufs=2))
        p2p = ctx.enter_context(tc.tile_pool(name="p2p", bufs=6))

        for h in range(NHEADS):
            # f2 biases for this head: [128, 4, 8] -> (p, i, c)
            f2a = p2s.tile([128, IT, NC], F32, tag="f2a")
            for c in range(NC):
                nc.sync.dma_start(
                    f2a[:, :, c],
                    f_out[c:c + 1, h * R:(h + 1) * R].rearrange(
                        "one (b p) -> (one p) b", p=128))
            b1 = p2s.tile([128, IT, NC], F32, tag="b1")
            nc.vector.tensor_scalar_add(b1, f2a, BIAS_LN)
            b2 = p2s.tile([128, IT, NC], F32, tag="b2")
            nc.vector.tensor_scalar(b2, f2a, ALPHA, BIAS_LN, OP.mult, OP.add)

            rs_acc = p2s.tile([128, R], F32, tag="rs_acc")
            nc.vector.memset(rs_acc, 0.0)

            hps = [pps.tile([128, R], F32, name=f"hps{h}_{os}", tag=f"h{os}")
                   for os in range(8)]
            for c in range(NC):
                wht4 = p2w.tile([128, IT, NHID], F16, tag="wht", bufs=3)
                nc.sync.dma_start(
                    wht4, ag1_out[h][c * 128:(c + 1) * 128, :].rearrange(
                        "p (i o) -> p i o", i=IT))
                if h > 0:
                    _elu_tail(c)
                for i in range(IT):
                    jt = c * IT + i
                    e1 = p2p.tile([128, R], BF16, tag="e1")
                    nc.scalar.activation(e1, f1b[h], AF.Exp,
                                         bias=b1[:, i, c:c + 1], scale=1.0)
                    e2 = p2p.tile([128, R], BF16, tag="e2")
                    nc.scalar.activation(e2, f1b[h], AF.Exp,
                                         bias=b2[:, i, c:c + 1], scale=ALPHA)
                    nc.vector.tensor_tensor(e1, e1, e2, OP.max)
                    u = p2p.tile([128, R], BF16, tag="u")
                    nc.vector.tensor_tensor(u, e1, adjT[:, jt, :], OP.mult)
                    nc.gpsimd.tensor_tensor(rs_acc, rs_acc, u, OP.add)
                    for os in range(8):
                        nc.tensor.matmul(
                            hps[os], lhsT=wht4[:, i, os * 128:(os + 1) * 128],
                            rhs=u, start=(jt == 0), stop=(jt == JT - 1))

            # plain-drain bank 0 so the rowsum matmul can take its slot
            h0sb = p2s.tile([128, R], F32, tag="h0sb")
            nc.vector.tensor_copy(h0sb, hps[0])
            rs_ps = pps.tile([1, R], F32, name=f"rs_ps{h}", tag="h0")
            nc.tensor.matmul(rs_ps, lhsT=ones_col, rhs=rs_acc, start=True, stop=True)
            recip = p2s.tile([1, R], F32, tag="recip")
            nc.vector.reciprocal(recip, rs_ps)
            bc_ps = pps.tile([128, R], F32, name=f"bc_ps{h}", tag="h0")
            nc.tensor.matmul(bc_ps, lhsT=ones_row, rhs=recip, start=True, stop=True)
            rb = p2s.tile([128, R], F32, tag="rb")
            nc.vector.tensor_copy(rb, bc_ps)

            hstage = p2s.tile([128, 8, R], F16, name=f"hstage{h}", tag="hstage",
                              bufs=1)
            for os in range(8):
                nc.vector.tensor_tensor(hstage[:, os, :],
                                        h0sb if os == 0 else hps[os], rb, OP.mult)


            def _elu_tail(os, h=h, hstage=hstage):
                mn = p2w.tile([128, R], F16, tag="u2f")
                nc.vector.tensor_scalar_min(mn, hstage[:, os, :], 0.0)
                ex = p2w.tile([128, R], F16, tag="ex")
                nc.scalar.activation(ex, mn, AF.Exp)
                nc.vector.scalar_tensor_tensor(
                    out=xcatT[:, h * 8 + os, :], in0=ex, scalar=-1.0,
                    in1=hstage[:, os, :], op0=OP.add, op1=OP.max)

        for os in range(8):
            _elu_tail(os)

        # =============== phase 3: Wh2 = x_cat @ W_out; g1/g2 ===============
        wh2T_ps = pps.tile([BIT, R], F32, tag="h2")
        for k in range(KT):
            nc.tensor.matmul(wh2T_ps, lhsT=wob[:, k, :], rhs=xcatT[:, k, :],
                             start=(k == 0), stop=(k == KT - 1))
        wh2T = p2c.tile([BIT, R], F32)
        nc.vector.tensor_copy(wh2T, wh2T_ps)
        g1T_ps = pps.tile([1, R], F32, tag="h3")
        nc.tensor.matmul(g1T_ps, lhsT=a1o_col, rhs=wh2T, start=True, stop=True)
        g1T = p2c.tile([1, R], F32)
        nc.vector.tensor_copy(g1T, g1T_ps)

        for i in range(IT):
            tp_ps = pps.tile([128, BIT], F32, name=f"w2t{i}", tag="h4")
            nc.tensor.transpose(tp_ps, wh2T[:, i * 128:(i + 1) * 128],
                                ident[:BIT, :BIT])
            wh2n = p2w.tile([128, BIT], F32, tag="wh2n")
            nc.vector.tensor_copy(wh2n, tp_ps)
            g2c = p2w.tile([128, 1], F32, tag="g2c")
            scratch2 = p2w.tile([128, BIT], F32, tag="scratch2")
            nc.vector.scalar_tensor_tensor(
                out=scratch2, in0=wh2n, scalar=0.0, in1=a2o_b,
                op0=OP.bypass, op1=OP.mult, accum_out=g2c)
            base = i * (BIT + 1)
            nc.sync.dma_start(ag2_in[:, base:base + BIT], wh2n)
            nc.sync.dma_start(ag2_in[:, base + BIT:base + BIT + 1], g2c)

        nc.gpsimd.collective_compute(
            "AllGather", OP.bypass, ins=[ag2_in.opt()], outs=[ag2_out.opt()],
            replica_groups=rg)

        # =============== phase 4: output attention ===============
        g1b_ps = pps.tile([128, R], F32, tag="h5")
        nc.tensor.matmul(g1b_ps, lhsT=ones_row, rhs=g1T, start=True, stop=True)
        g1b = p2c.tile([128, R], F32)
        nc.vector.tensor_copy(g1b, g1b_ps)

        # 4-way accumulator tree keeps the serial GpSimd chain short
        rs2_acc = [p2s.tile([128, R], F32, name=f"rs2_{a}", tag=f"rs2_{a}")
                   for a in range(4)]
        for a in range(4):
            nc.vector.memset(rs2_acc[a], 0.0)
        ht2_ps = pps.tile([BIT, R], F32, tag="h6")
        for c in range(NC):
            w2t4 = p2w.tile([128, IT, BIT + 1], F32, tag="w2t4")
            nc.sync.dma_start(
                w2t4, ag2_out[c * 128:(c + 1) * 128, :].rearrange(
                    "p (i z) -> p i z", i=IT))
            g2s4 = p2w.tile([128, IT], F32, tag="g2s4")
            nc.vector.tensor_scalar_mul(g2s4, w2t4[:, :, BIT], ALPHA)
            w2b = p2w.tile([128, IT, BIT], F16, tag="w2b")
            nc.vector.tensor_copy(w2b, w2t4[:, :, :BIT])
            for i in range(IT):
                jt = c * IT + i
                e1 = p2p.tile([128, R], BF16, tag="e1")
                nc.scalar.activation(e1, g1b, AF.Exp,
                                     bias=w2t4[:, i, BIT:BIT + 1], scale=1.0)
                e2 = p2p.tile([128, R], BF16, tag="e2")
                nc.scalar.activation(e2, g1b, AF.Exp,
                                     bias=g2s4[:, i:i + 1], scale=ALPHA)
                nc.vector.tensor_tensor(e1, e1, e2, OP.max)
                u2 = p2p.tile([128, R], BF16, tag="u")
                nc.vector.tensor_tensor(u2, e1, adjT[:, jt, :], OP.mult)
                nc.gpsimd.tensor_tensor(rs2_acc[jt % 4], rs2_acc[jt % 4], u2,
                                        OP.add)
                nc.tensor.matmul(ht2_ps, lhsT=w2b[:, i, :], rhs=u2,
                                 start=(jt == 0), stop=(jt == JT - 1))

        nc.vector.tensor_tensor(rs2_acc[0], rs2_acc[0], rs2_acc[1], OP.add)
        nc.vector.tensor_tensor(rs2_acc[2], rs2_acc[2], rs2_acc[3], OP.add)
        nc.vector.tensor_tensor(rs2_acc[0], rs2_acc[0], rs2_acc[2], OP.add)
        rs2_ps = pps.tile([1, R], F32, tag="h7")
        nc.tensor.matmul(rs2_ps, lhsT=ones_col, rhs=rs2_acc[0], start=True, stop=True)
        recip2 = p2c.tile([1, R], F32)
        nc.vector.reciprocal(recip2, rs2_ps)
        bc2_ps = pps.tile([128, R], F32, tag="h0")
        nc.tensor.matmul(bc2_ps, lhsT=ones_row, rhs=recip2, start=True, stop=True)
        rb2 = p2c.tile([128, R], F32)
        nc.vector.tensor_copy(rb2, bc2_ps)

        ot = p2c.tile([BIT, R], F32)
        nc.vector.tensor_tensor(ot, ht2_ps, rb2[:BIT, :], OP.mult)
        outT = p2c.tile([BIT, R], F32)
        nc.scalar.activation(outT, ot, AF.Tanh)
        for i in range(IT):
            tp_ps = pps.tile([128, BIT], F32, name=f"ot{i}", tag="h1")
            nc.tensor.transpose(tp_ps, outT[:, i * 128:(i + 1) * 128],
                                ident[:BIT, :BIT])
            ob = p2w.tile([128, BIT], F32, tag="ob")
            nc.vector.tensor_copy(ob, tp_ps)
            nc.sync.dma_start(out_d[i * 128:(i + 1) * 128, :], ob)

    _split_excess_waits(nc, max_waits=1)
    return nc


_CACHED = None


def _get_program():
    global _CACHED
    if _CACHED is None:
        _CACHED = build_program()
    return _CACHED


def _interleave(a, kt):
    """[kt*128, free...] -> [128, kt, free...] partition-major."""
    return np.ascontiguousarray(
        a.reshape(kt, 128, *a.shape[1:]).transpose(1, 0, *range(2, a.ndim + 1)))


def make_in_maps(x, adj, W, a1, a2, W_out, a1_out, a2_out):
    import ml_dtypes
    xT = np.ascontiguousarray(x.T)
    adjT_bf = adj.T.astype(ml_dtypes.bfloat16)
    # B = [W[h] @ a1[h] (4 cols) | W[h] @ a2[h] (4 cols)]  (fp32 logit vecs)
    B = np.concatenate(
        [np.stack([W[h] @ a1[h] for h in range(NHEADS)], axis=1),
         np.stack([W[h] @ a2[h] for h in range(NHEADS)], axis=1)],
        axis=1).astype(np.float32)
    # W interleaved: [h, 128, KT, NHID]
    W_sh = np.ascontiguousarray(
        W.reshape(NHEADS, KT, 128, NHID).transpose(0, 2, 1, 3)
).astype(np.float16)
    B_sh = _interleave(B, KT)
    Wo_sh = _interleave(W_out, KT)
    in_maps = []
    for d in range(NC):
        cols = slice(d * R, (d + 1) * R)
        in_maps.append({
            "x_sh": _interleave(np.ascontiguousarray(xT[:, cols]), KT),
            "W_sh": W_sh,
            "B_sh": B_sh,
            "adj_sh": _interleave(np.ascontiguousarray(adjT_bf[:, cols]), JT),
            "Wo_sh": Wo_sh,
            "a1_out": a1_out, "a2_out": a2_out,
        })
    return in_maps


def kernel(x, adj, W, a1, a2, W_out, a1_out, a2_out, _trace=False):
    nc = _get_program()
    in_maps = make_in_maps(np.asarray(x, np.float32), np.asarray(adj, np.float32),
                           np.asarray(W, np.float32), np.asarray(a1, np.float32),
                           np.asarray(a2, np.float32), np.asarray(W_out, np.float32),
                           np.asarray(a1_out, np.float32),
                           np.asarray(a2_out, np.float32))
    res = bass_utils.run_bass_kernel_spmd(
        nc, in_maps, core_ids=list(range(NC)), trace=_trace)
    out = np.concatenate([res.results[d]["out_rows"] for d in range(NC)], axis=0)
    if _trace:
        kernel.last_exec_time_ns = res.exec_time_ns
        kernel.last_results = res
    return out


# revision 31
# speedup vs baseline: 1.0027x; 1.0027x over previous
"""Trainium2 Bass kernel for nn_GATNet_IMG (dense 2-layer GAT, N=4096).

Sharding: 1D row-parallel over the node dim across 8 NeuronCores.
Each core computes Wh for its 512 rows (all 4 heads), AllGathers Wh
per head (overlapped with the next head's matmuls), then computes its
[512, 4096] attention block per head with a fused masked softmax (no
NxN matrix ever hits HBM), aggregates h^T = Wh^T @ u on TensorE, and
repeats the same pattern for the output attention layer.

Key tricks:
  - exp(leaky_relu(s)) == max(exp(s), exp(0.2*s))  (exact, all s)
  - attention logit halves f1/f2 come from one tiny [8]-wide matmul
    x @ B where B = [W[h]@a1[h] | W[h]@a2[h]] is host-precomputed, so
    logit precision is independent of the big-GEMM compute dtype
  - unnormalized attention + row-sum via ones-matmul; normalization is
    a post-scale of h^T (free-axis scale via a broadcast matmul)
  - elu(x) == max(exp(min(x, 0)) - 1, x)           (exact)
  - ln(2^-30) folded into the exp bias keeps row sums in DVE
    reciprocal range
  - host-side sharding pre-transposes x/adj AND interleaves all heavy
    streams into [128, ktiles, free] partition-major layouts so every
    DMA moves 8-64KB contiguous per partition (packet-rate limit)
"""

import math
from contextlib import ExitStack

import numpy as np

import concourse.bass as bass
import concourse.mybir as mybir
import concourse.tile as tile
from concourse import bass_utils
from concourse.masks import make_identity

F32 = mybir.dt.float32
F32R = mybir.dt.float32r
F16 = mybir.dt.float16
BF16 = mybir.dt.bfloat16
AF = mybir.ActivationFunctionType
OP = mybir.AluOpType

N = 4096
NFEAT = 4096
NHID = 1024
NHEADS = 4
BIT = 64
NC = 8
R = N // NC          # 512 attention rows per core
KT = NFEAT // 128    # 32 k tiles
JT = N // 128        # 32 node-column tiles
IT = R // 128        # 4 row tiles per core
ALPHA = 0.2
BIAS_LN = -30.0 * math.log(2.0)   # ln(2^-30) folded into exp bias (layer 1)


def _split_excess_waits(nc, max_waits=1):
    """walrus codegen rejects >max_waits sync-wait commands per instruction;
    push excess waits onto preceding same-engine NoOps."""
    n_fixed = 0
    for f in nc.m.functions:
        for b in f.blocks:
            new_insts = []
            changed = False
            for inst in b.instructions:
                si = getattr(inst, "sync_info", None)
                if si is not None and si.on_wait and len(si.on_wait) > max_waits:
                    waits = list(si.on_wait)
                    excess, keep = waits[:-max_waits], waits[-max_waits:]
                    for ci in range(0, len(excess), max_waits):
                        nop = mybir.InstNoOp(
                            name=f"{inst.name}-ws{ci}",
                            sync_info=mybir.SyncInfo(
                                on_wait=excess[ci:ci + max_waits], on_update=[]
                            ),
                            bass_nofuse=True,
                            engine=inst.engine,
                        )
                        new_insts.append(nop)
                    inst.sync_info = mybir.SyncInfo(
                        on_wait=keep, on_update=list(si.on_update or [])
                    )
                    n_fixed += 1
                    changed = True
                new_insts.append(inst)
            if changed:
                insts = b.instructions
                try:
                    b.instructions = new_insts
                except Exception:
                    while len(insts):
                        insts.pop()
                    for i in new_insts:
                        insts.append(i)
    return n_fixed


def build_program():
    nc = bass.Bass("TRN2", target_bir_lowering=False, debug=False, num_devices=NC)

    # host-interleaved inputs: [128 partitions, ktiles, free]
    x_d = nc.dram_tensor("x_sh", [128, KT, R], F32, kind="ExternalInput").ap()
    W_d = nc.dram_tensor("W_sh", [NHEADS, 128, KT, NHID], F16,
                         kind="ExternalInput").ap()
    B_d = nc.dram_tensor("B_sh", [128, KT, 2 * NHEADS], F32,
                         kind="ExternalInput").ap()
    adj_d = nc.dram_tensor("adj_sh", [128, JT, R], BF16, kind="ExternalInput").ap()
    wo_d = nc.dram_tensor("Wo_sh", [128, KT, BIT], F32, kind="ExternalInput").ap()
    a1o_d = nc.dram_tensor("a1_out", [BIT], F32, kind="ExternalInput").ap()
    a2o_d = nc.dram_tensor("a2_out", [BIT], F32, kind="ExternalInput").ap()
    out_d = nc.dram_tensor("out_rows", [R, BIT], F32, kind="ExternalOutput").ap()

    # collective bounce buffers; Wh per head so each head's allgather overlaps
    # the next head's phase-1 compute. Layout: [128, i*NHID] per rank so the
    # gathered output reads back with 8KB/partition lines.
    ag1_in = [nc.dram_tensor(f"ag1_in{h}", [128, IT * NHID], F16).ap()
              for h in range(NHEADS)]
    ag1_out = [nc.dram_tensor(f"ag1_out{h}", [NC * 128, IT * NHID], F16,
                              addr_space="Shared").ap() for h in range(NHEADS)]
    f_in = nc.dram_tensor("f_in", [1, NHEADS * R], F32).ap()
    f_out = nc.dram_tensor("f_out", [NC, NHEADS * R], F32, addr_space="Shared").ap()
    ag2_in = nc.dram_tensor("ag2_in", [128, IT * (BIT + 1)], F32).ap()
    ag2_out = nc.dram_tensor("ag2_out", [NC * 128, IT * (BIT + 1)], F32,
                             addr_space="Shared").ap()

    rg = [list(range(NC))]

    with tile.TileContext(nc) as tc, ExitStack() as ctx:
        cp = ctx.enter_context(tc.tile_pool(name="const", bufs=1))
        ident = cp.tile([128, 128], F32)
        make_identity(nc, ident)
        ones_col = cp.tile([128, 1], F32)
        nc.vector.memset(ones_col, 1.0)
        ones_row = cp.tile([1, 128], F32)
        nc.vector.memset(ones_row, 1.0)
        a1o_col = cp.tile([BIT, 1], F32)
        nc.sync.dma_start(a1o_col, a1o_d.rearrange("(b one) -> b one", one=1))
        a2o_b = cp.tile([128, BIT], F32)
        nc.sync.dma_start(
            a2o_b, a2o_d.rearrange("(one b) -> one b", one=1).to_broadcast([128, BIT]))
        # adjacency mask, resident for both attention layers (one big DMA)
        adjT = cp.tile([128, JT, R], BF16)
        nc.sync.dma_start(adjT, adj_d)
        # f1 broadcast tiles [128, R] per head
        f1b = [cp.tile([128, R], F32, name=f"f1b_{h}") for h in range(NHEADS)]

        # =============== phase 0: f-logit halves via x @ B ===============
        with tc.tile_pool(name="p0", bufs=1) as p0:
            with tc.tile_pool(name="p0ps", bufs=1, space="PSUM") as p0ps:
                bres = p0.tile([128, KT, 2 * NHEADS], F32R)
                nc.sync.dma_start(bres, B_d.bitcast(F32R))
                xres = p0.tile([128, KT, R], F32R)
                nc.sync.dma_start(xres, x_d.bitcast(F32R))
                bps = p0ps.tile([2 * NHEADS, R], F32)
                for k in range(KT):
                    nc.tensor.matmul(bps, lhsT=bres[:, k, :], rhs=xres[:, k, :],
                                     start=(k == 0), stop=(k == KT - 1))
                # f2 of my rows, all heads -> allgather (concat on partitions)
                fall = p0.tile([2 * NHEADS, R], F32)
                nc.vector.tensor_copy(fall, bps)
                nc.sync.dma_start(
                    f_in.rearrange("one (h j) -> (one h) j", h=NHEADS),
                    fall[NHEADS:, :])
                nc.gpsimd.collective_compute(
                    "AllGather", OP.bypass, ins=[f_in.opt()], outs=[f_out.opt()],
                    replica_groups=rg)
                for h in range(NHEADS):
                    # row h -> partition 0 (SBUF->SBUF DMA), then broadcast
                    f1row = p0.tile([1, R], F32, name=f"f1row{h}", tag="f1row")
                    nc.sync.dma_start(f1row, fall[h:h + 1, :])
                    fb_ps = p0ps.tile([128, R], F32, name=f"fb_ps{h}", tag="fbps")
                    nc.tensor.matmul(fb_ps, lhsT=ones_row, rhs=f1row,
                                     start=True, stop=True)
                    nc.vector.tensor_copy(f1b[h], fb_ps)

            # =============== phase 1: Wh = x @ W[h] ===============
            with tc.tile_pool(name="p1s", bufs=2) as p1s, \
                 tc.tile_pool(name="p1ps", bufs=1, space="PSUM") as p1ps, \
                 tc.tile_pool(name="p1d", bufs=2) as p1d:
                xp1 = p0.tile([128, KT, R], F16)
                nc.vector.tensor_copy(xp1, xres.bitcast(F32))
                for h in range(NHEADS):
                    ps = [[p1ps.tile([128, 512], F32, name=f"ps_{h}_{i}_{oh}",
                                     tag=f"ps{i}{oh}") for oh in range(2)]
                          for i in range(IT)]
                    for kb in range(4):
                        wres = p1s.tile([128, 8, NHID], F16, tag="wres")
                        nc.sync.dma_start(
                            wres, W_d[h, :, kb * 8:(kb + 1) * 8, :])
                        for kk in range(8):
                            k = kb * 8 + kk
                            for i in range(IT):
                                for oh in range(2):
                                    nc.tensor.matmul(
                                        ps[i][oh],
                                        lhsT=xp1[:, k, i * 128:(i + 1) * 128],
                                        rhs=wres[:, kk, oh * 512:(oh + 1) * 512],
                                        start=(k == 0), stop=(k == KT - 1),
                                    )
                    for i in range(IT):
                        wh_sb = p1d.tile([128, NHID], F16, tag="wh_sb")
                        nc.vector.tensor_copy(wh_sb[:, :512], ps[i][0])
                        nc.vector.tensor_copy(wh_sb[:, 512:], ps[i][1])
                        nc.sync.dma_start(
                            ag1_in[h][:, i * NHID:(i + 1) * NHID], wh_sb)
                    # allgather this head's Wh while later heads compute
                    nc.gpsimd.collective_compute(
                        "AllGather", OP.bypass, ins=[ag1_in[h].opt()],
                        outs=[ag1_out[h].opt()], replica_groups=rg)

        # =============== phase 2: attention + aggregate, per head ===============
        p2c = ctx.enter_context(tc.tile_pool(name="p2c", bufs=1))
        xcatT = p2c.tile([128, KT, R], F16)
        wof = p2c.tile([128, KT, BIT], F32)
        nc.sync.dma_start(wof, wo_d)
        wob = p2c.tile([128, KT, BIT], F16)
        nc.vector.tensor_copy(wob, wof)

        pps = ctx.enter_context(tc.tile_pool(name="pps", bufs=1, space="PSUM"))
        p2s = ctx.enter_context(tc.tile_pool(name="p2s", bufs=2))
        p2w = ctx.enter_context(tc.tile_pool(name="p2w", bufs=2))
        p2p = ctx.enter_context(tc.tile_pool(name="p2p", bufs=4))

        for h in range(NHEADS):
            # f2 biases for this head: [128, 4, 8] -> (p, i, c)
            f2a = p2s.tile([128, IT, NC], F32, tag="f2a")
            for c in range(NC):
                nc.sync.dma_start(
                    f2a[:, :, c],
                    f_out[c:c + 1, h * R:(h + 1) * R].rearrange(
                        "one (b p) -> (one p) b", p=128))
            b1 = p2s.tile([128, IT, NC], F32, tag="b1")
            nc.vector.tensor_scalar_add(b1, f2a, BIAS_LN)
            b2 = p2s.tile([128, IT, NC], F32, tag="b2")
            nc.vector.tensor_scalar(b2, f2a, ALPHA, BIAS_LN, OP.mult, OP.add)

            rs_acc = p2s.tile([128, R], F32, tag="rs_acc")
            nc.vector.memset(rs_acc, 0.0)

            hps = [pps.tile([128, R], F32, name=f"hps{h}_{os}", tag=f"h{os}")
                   for os in range(8)]
            for c in range(NC):
                wht4 = p2w.tile([128, IT, NHID], F16, tag="wht", bufs=3)
                nc.sync.dma_start(
                    wht4, ag1_out[h][c * 128:(c + 1) * 128, :].rearrange(
                        "p (i o) -> p i o", i=IT))
                if h > 0:
                    _elu_tail(c)
                for i in range(IT):
                    jt = c * IT + i
                    e1 = p2p.tile([128, R], BF16, tag="e1")
                    nc.scalar.activation(e1, f1b[h], AF.Exp,
                                         bias=b1[:, i, c:c + 1], scale=1.0)
                    e2 = p2p.tile([128, R], BF16, tag="e2")
                    nc.scalar.activation(e2, f1b[h], AF.Exp,
                                         bias=b2[:, i, c:c + 1], scale=ALPHA)
                    nc.vector.tensor_tensor(e1, e1, e2, OP.max)
                    u = p2p.tile([128, R], BF16, tag="u")
                    nc.vector.tensor_tensor(u, e1, adjT[:, jt, :], OP.mult)
                    nc.gpsimd.tensor_tensor(rs_acc, rs_acc, u, OP.add)
                    for os in range(8):
                        nc.tensor.matmul(
                            hps[os], lhsT=wht4[:, i, os * 128:(os + 1) * 128],
                            rhs=u, start=(jt == 0), stop=(jt == JT - 1))

            # plain-drain bank 0 so the rowsum matmul can take its slot
            h0sb = p2s.tile([128, R], F32, tag="h0sb")
            nc.vector.tensor_copy(h0sb, hps[0])
            rs_ps = pps.tile([1, R], F32, name=f"rs_ps{h}", tag="h0")
            nc.tensor.matmul(rs_ps, lhsT=ones_col, rhs=rs_acc, start=True, stop=True)
            recip = p2s.tile([1, R], F32, tag="recip")
            nc.vector.reciprocal(recip, rs_ps)
            bc_ps = pps.tile([128, R], F32, name=f"bc_ps{h}", tag="h0")
            nc.tensor.matmul(bc_ps, lhsT=ones_row, rhs=recip, start=True, stop=True)
            rb = p2s.tile([128, R], F32, tag="rb")
            nc.vector.tensor_copy(rb, bc_ps)

            hstage = p2s.tile([128, 8, R], F16, name=f"hstage{h}", tag="hstage",
                              bufs=1)
            for os in range(8):
                nc.vector.tensor_tensor(hstage[:, os, :],
                                        h0sb if os == 0 else hps[os], rb, OP.mult)


            def _elu_tail(os, h=h, hstage=hstage):
                mn = p2w.tile([128, R], F16, tag="u2f")
                nc.vector.tensor_scalar_min(mn, hstage[:, os, :], 0.0)
                ex = p2w.tile([128, R], F16, tag="ex")
                nc.scalar.activation(ex, mn, AF.Exp)
                nc.vector.scalar_tensor_tensor(
                    out=xcatT[:, h * 8 + os, :], in0=ex, scalar=-1.0,
                    in1=hstage[:, os, :], op0=OP.add, op1=OP.max)

        for os in range(8):
            _elu_tail(os)

        # =============== phase 3: Wh2 = x_cat @ W_out; g1/g2 ===============
        wh2T_ps = pps.tile([BIT, R], F32, tag="h2")
        for k in range(KT):
            nc.tensor.matmul(wh2T_ps, lhsT=wob[:, k, :], rhs=xcatT[:, k, :],
                             start=(k == 0), stop=(k == KT - 1))
        wh2T = p2c.tile([BIT, R], F32)
        nc.vector.tensor_copy(wh2T, wh2T_ps)
        g1T_ps = pps.tile([1, R], F32, tag="h3")
        nc.tensor.matmul(g1T_ps, lhsT=a1o_col, rhs=wh2T, start=True, stop=True)
        g1T = p2c.tile([1, R], F32)
        nc.vector.tensor_copy(g1T, g1T_ps)

        for i in range(IT):
            tp_ps = pps.tile([128, BIT], F32, name=f"w2t{i}", tag="h4")
            nc.tensor.transpose(tp_ps, wh2T[:, i * 128:(i + 1) * 128],
                                ident[:BIT, :BIT])
            wh2n = p2w.tile([128, BIT], F32, tag="wh2n")
            nc.vector.tensor_copy(wh2n, tp_ps)
            g2c = p2w.tile([128, 1], F32, tag="g2c")
            scratch2 = p2w.tile([128, BIT], F32, tag="scratch2")
            nc.vector.scalar_tensor_tensor(
                out=scratch2, in0=wh2n, scalar=0.0, in1=a2o_b,
                op0=OP.bypass, op1=OP.mult, accum_out=g2c)
            base = i * (BIT + 1)
            nc.sync.dma_start(ag2_in[:, base:base + BIT], wh2n)
            nc.sync.dma_start(ag2_in[:, base + BIT:base + BIT + 1], g2c)

        nc.gpsimd.collective_compute(
            "AllGather", OP.bypass, ins=[ag2_in.opt()], outs=[ag2_out.opt()],
            replica_groups=rg)

        # =============== phase 4: output attention ===============
        g1b_ps = pps.tile([128, R], F32, tag="h5")
        nc.tensor.matmul(g1b_ps, lhsT=ones_row, rhs=g1T, start=True, stop=True)
        g1b = p2c.tile([128, R], F32)
        nc.vector.tensor_copy(g1b, g1b_ps)

        # 4-way accumulator tree keeps the serial GpSimd chain short
        rs2_acc = [p2s.tile([128, R], F32, name=f"rs2_{a}", tag=f"rs2_{a}")
                   for a in range(4)]
        for a in range(4):
            nc.vector.memset(rs2_acc[a], 0.0)
        ht2_ps = pps.tile([BIT, R], F32, tag="h6")
        for c in range(NC):
            w2t4 = p2w.tile([128, IT, BIT + 1], F32, tag="w2t4")
            nc.sync.dma_start(
                w2t4, ag2_out[c * 128:(c + 1) * 128, :].rearrange(
                    "p (i z) -> p i z", i=IT))
            g2s4 = p2w.tile([128, IT], F32, tag="g2s4")
            nc.vector.tensor_scalar_mul(g2s4, w2t4[:, :, BIT], ALPHA)
            w2b = p2w.tile([128, IT, BIT], F16, tag="w2b")
            nc.vector.tensor_copy(w2b, w2t4[:, :, :BIT])
            for i in range(IT):
                jt = c * IT + i
                e1 = p2p.tile([128, R], BF16, tag="e1")
                nc.scalar.activation(e1, g1b, AF.Exp,
                                     bias=w2t4[:, i, BIT:BIT + 1], scale=1.0)
                e2 = p2p.tile([128, R], BF16, tag="e2")
                nc.scalar.activation(e2, g1b, AF.Exp,
                                     bias=g2s4[:, i:i + 1], scale=ALPHA)
                nc.vector.tensor_tensor(e1, e1, e2, OP.max)
                u2 = p2p.tile([128, R], BF16, tag="u")
                nc.vector.tensor_tensor(u2, e1, adjT[:, jt, :], OP.mult)
                nc.gpsimd.tensor_tensor(rs2_acc[jt % 4], rs2_acc[jt % 4], u2,
                                        OP.add)
                nc.tensor.matmul(ht2_ps, lhsT=w2b[:, i, :], rhs=u2,
                                 start=(jt == 0), stop=(jt == JT - 1))

        nc.vector.tensor_tensor(rs2_acc[0], rs2_acc[0], rs2_acc[1], OP.add)
        nc.vector.tensor_tensor(rs2_acc[2], rs2_acc[2], rs2_acc[3], OP.add)
        nc.vector.tensor_tensor(rs2_acc[0], rs2_acc[0], rs2_acc[2], OP.add)
        rs2_ps = pps.tile([1, R], F32, tag="h7")
        nc.tensor.matmul(rs2_ps, lhsT=ones_col, rhs=rs2_acc[0], start=True, stop=True)
        recip2 = p2c.tile([1, R], F32)
        nc.vector.reciprocal(recip2, rs2_ps)
        bc2_ps = pps.tile([128, R], F32, tag="h0")
        nc.tensor.matmul(bc2_ps, lhsT=ones_row, rhs=recip2, start=True, stop=True)
        rb2 = p2c.tile([128, R], F32)
        nc.vector.tensor_copy(rb2, bc2_ps)

        ot = p2c.tile([BIT, R], F32)
        nc.vector.tensor_tensor(ot, ht2_ps, rb2[:BIT, :], OP.mult)
        outT = p2c.tile([BIT, R], F32)
        nc.scalar.activation(outT, ot, AF.Tanh)
        for i in range(IT):
            tp_ps = pps.tile([128, BIT], F32, name=f"ot{i}", tag="h1")
            nc.tensor.transpose(tp_ps, outT[:, i * 128:(i + 1) * 128],
                                ident[:BIT, :BIT])
            ob = p2w.tile([128, BIT], F32, tag="ob")
            nc.vector.tensor_copy(ob, tp_ps)
            nc.sync.dma_start(out_d[i * 128:(i + 1) * 128, :], ob)

    _split_excess_waits(nc, max_waits=1)
    return nc


_CACHED = None


def _get_program():
    global _CACHED
    if _CACHED is None:
        _CACHED = build_program()
    return _CACHED


def _interleave(a, kt):
    """[kt*128, free...] -> [128, kt, free...] partition-major."""
    return np.ascontiguousarray(
        a.reshape(kt, 128, *a.shape[1:]).transpose(1, 0, *range(2, a.ndim + 1)))


def make_in_maps(x, adj, W, a1, a2, W_out, a1_out, a2_out):
    import ml_dtypes
    xT = np.ascontiguousarray(x.T)
    adjT_bf = adj.T.astype(ml_dtypes.bfloat16)
    # B = [W[h] @ a1[h] (4 cols) | W[h] @ a2[h] (4 cols)]  (fp32 logit vecs)
    B = np.concatenate(
        [np.stack([W[h] @ a1[h] for h in range(NHEADS)], axis=1),
         np.stack([W[h] @ a2[h] for h in range(NHEADS)], axis=1)],
        axis=1).astype(np.float32)
    # W interleaved: [h, 128, KT, NHID]
    W_sh = np.ascontiguousarray(
        W.reshape(NHEADS, KT, 128, NHID).transpose(0, 2, 1, 3)
).astype(np.float16)
    B_sh = _interleave(B, KT)
    Wo_sh = _interleave(W_out, KT)
    in_maps = []
    for d in range(NC):
        cols = slice(d * R, (d + 1) * R)
        in_maps.append({
            "x_sh": _interleave(np.ascontiguousarray(xT[:, cols]), KT),
            "W_sh": W_sh,
            "B_sh": B_sh,
            "adj_sh": _interleave(np.ascontiguousarray(adjT_bf[:, cols]), JT),
            "Wo_sh": Wo_sh,
            "a1_out": a1_out, "a2_out": a2_out,
        })
    return in_maps


def kernel(x, adj, W, a1, a2, W_out, a1_out, a2_out, _trace=False):
    nc = _get_program()
    in_maps = make_in_maps(np.asarray(x, np.float32), np.asarray(adj, np.float32),
                           np.asarray(W, np.float32), np.asarray(a1, np.float32),
                           np.asarray(a2, np.float32), np.asarray(W_out, np.float32),
                           np.asarray(a1_out, np.float32),
                           np.asarray(a2_out, np.float32))
    res = bass_utils.run_bass_kernel_spmd(
        nc, in_maps, core_ids=list(range(NC)), trace=_trace)
    out = np.concatenate([res.results[d]["out_rows"] for d in range(NC)], axis=0)
    if _trace:
        kernel.last_exec_time_ns = res.exec_time_ns
        kernel.last_results = res
    return out


# revision 32
# speedup vs baseline: 1.0102x; 1.0075x over previous
"""Trainium2 Bass kernel for nn_GATNet_IMG (dense 2-layer GAT, N=4096).

Sharding: 1D row-parallel over the node dim across 8 NeuronCores.
Each core computes Wh for its 512 rows (all 4 heads), AllGathers Wh
per head (overlapped with the next head's matmuls), then computes its
[512, 4096] attention block per head with a fused masked softmax (no
NxN matrix ever hits HBM), aggregates h^T = Wh^T @ u on TensorE, and
repeats the same pattern for the output attention layer.

Key tricks:
  - exp(leaky_relu(s)) == max(exp(s), exp(0.2*s))  (exact, all s)
  - attention logit halves f1/f2 come from one tiny [8]-wide matmul
    x @ B where B = [W[h]@a1[h] | W[h]@a2[h]] is host-precomputed, so
    logit precision is independent of the big-GEMM compute dtype
  - unnormalized attention + row-sum via ones-matmul; normalization is
    a post-scale of h^T (free-axis scale via a broadcast matmul)
  - elu(x) == max(exp(min(x, 0)) - 1, x)           (exact)
  - ln(2^-30) folded into the exp bias keeps row sums in DVE
    reciprocal range
  - host-side sharding pre-transposes x/adj AND interleaves all heavy
    streams into [128, ktiles, free] partition-major layouts so every
    DMA moves 8-64KB contiguous per partition (packet-rate limit)
"""

import math
from contextlib import ExitStack

import numpy as np

import concourse.bass as bass
import concourse.mybir as mybir
import concourse.tile as tile
from concourse import bass_utils
from concourse.masks import make_identity

F32 = mybir.dt.float32
F32R = mybir.dt.float32r
F16 = mybir.dt.float16
BF16 = mybir.dt.bfloat16
AF = mybir.ActivationFunctionType
OP = mybir.AluOpType

N = 4096
NFEAT = 4096
NHID = 1024
NHEADS = 4
BIT = 64
NC = 8
R = N // NC          # 512 attention rows per core
KT = NFEAT // 128    # 32 k tiles
JT = N // 128        # 32 node-column tiles
IT = R // 128        # 4 row tiles per core
ALPHA = 0.2
BIAS_LN = -30.0 * math.log(2.0)   # ln(2^-30) folded into exp bias (layer 1)


def _split_excess_waits(nc, max_waits=1):
    """walrus codegen rejects >max_waits sync-wait commands per instruction;
    push excess waits onto preceding same-engine NoOps."""
    n_fixed = 0
    for f in nc.m.functions:
        for b in f.blocks:
            new_insts = []
            changed = False
            for inst in b.instructions:
                si = getattr(inst, "sync_info", None)
                if si is not None and si.on_wait and len(si.on_wait) > max_waits:
                    waits = list(si.on_wait)
                    excess, keep = waits[:-max_waits], waits[-max_waits:]
                    for ci in range(0, len(excess), max_waits):
                        nop = mybir.InstNoOp(
                            name=f"{inst.name}-ws{ci}",
                            sync_info=mybir.SyncInfo(
                                on_wait=excess[ci:ci + max_waits], on_update=[]
                            ),
                            bass_nofuse=True,
                            engine=inst.engine,
                        )
                        new_insts.append(nop)
                    inst.sync_info = mybir.SyncInfo(
                        on_wait=keep, on_update=list(si.on_update or [])
                    )
                    n_fixed += 1
                    changed = True
                new_insts.append(inst)
            if changed:
                insts = b.instructions
                try:
                    b.instructions = new_insts
                except Exception:
                    while len(insts):
                        insts.pop()
                    for i in new_insts:
                        insts.append(i)
    return n_fixed


def build_program():
    nc = bass.Bass("TRN2", target_bir_lowering=False, debug=False, num_devices=NC)

    # host-interleaved inputs: [128 partitions, ktiles, free]
    x_d = nc.dram_tensor("x_sh", [128, KT, R], F32, kind="ExternalInput").ap()
    W_d = nc.dram_tensor("W_sh", [NHEADS, 128, KT, NHID], F16,
                         kind="ExternalInput").ap()
    B_d = nc.dram_tensor("B_sh", [128, KT, 2 * NHEADS], F32,
                         kind="ExternalInput").ap()
    adj_d = nc.dram_tensor("adj_sh", [128, JT, R], BF16, kind="ExternalInput").ap()
    wo_d = nc.dram_tensor("Wo_sh", [128, KT, BIT], F32, kind="ExternalInput").ap()
    a1o_d = nc.dram_tensor("a1_out", [BIT], F32, kind="ExternalInput").ap()
    a2o_d = nc.dram_tensor("a2_out", [BIT], F32, kind="ExternalInput").ap()
    out_d = nc.dram_tensor("out_rows", [R, BIT], F32, kind="ExternalOutput").ap()

    # collective bounce buffers; Wh per head so each head's allgather overlaps
    # the next head's phase-1 compute. Layout: [128, i*NHID] per rank so the
    # gathered output reads back with 8KB/partition lines.
    ag1_in = [nc.dram_tensor(f"ag1_in{h}", [128, IT * NHID], F16).ap()
              for h in range(NHEADS)]
    ag1_out = [nc.dram_tensor(f"ag1_out{h}", [NC * 128, IT * NHID], F16,
                              addr_space="Shared").ap() for h in range(NHEADS)]
    f_in = nc.dram_tensor("f_in", [1, NHEADS * R], F32).ap()
    f_out = nc.dram_tensor("f_out", [NC, NHEADS * R], F32, addr_space="Shared").ap()
    ag2_in = nc.dram_tensor("ag2_in", [128, IT * (BIT + 1)], F32).ap()
    ag2_out = nc.dram_tensor("ag2_out", [NC * 128, IT * (BIT + 1)], F32,
                             addr_space="Shared").ap()

    rg = [list(range(NC))]

    with tile.TileContext(nc) as tc, ExitStack() as ctx:
        cp = ctx.enter_context(tc.tile_pool(name="const", bufs=1))
        ident = cp.tile([128, 128], F32)
        make_identity(nc, ident)
        ones_col = cp.tile([128, 1], F32)
        nc.vector.memset(ones_col, 1.0)
        ones_row = cp.tile([1, 128], F32)
        nc.vector.memset(ones_row, 1.0)
        a1o_col = cp.tile([BIT, 1], F32)
        nc.sync.dma_start(a1o_col, a1o_d.rearrange("(b one) -> b one", one=1))
        a2o_b = cp.tile([128, BIT], F32)
        nc.sync.dma_start(
            a2o_b, a2o_d.rearrange("(one b) -> one b", one=1).to_broadcast([128, BIT]))
        # adjacency mask, resident for both attention layers (one big DMA)
        adjT = cp.tile([128, JT, R], BF16)
        nc.sync.dma_start(adjT, adj_d)
        # f1 broadcast tiles [128, R] per head
        f1b = [cp.tile([128, R], F32, name=f"f1b_{h}") for h in range(NHEADS)]

        # =============== phase 0: f-logit halves via x @ B ===============
        with tc.tile_pool(name="p0", bufs=1) as p0:
            with tc.tile_pool(name="p0ps", bufs=1, space="PSUM") as p0ps:
                bres = p0.tile([128, KT, 2 * NHEADS], F32R)
                nc.sync.dma_start(bres, B_d.bitcast(F32R))
                xres = p0.tile([128, KT, R], F32R)
                for q in range(4):
                    nc.sync.dma_start(xres[:, q * 8:(q + 1) * 8, :],
                                      x_d[:, q * 8:(q + 1) * 8, :].bitcast(F32R))
                bps = p0ps.tile([2 * NHEADS, R], F32)
                for k in range(KT):
                    nc.tensor.matmul(bps, lhsT=bres[:, k, :], rhs=xres[:, k, :],
                                     start=(k == 0), stop=(k == KT - 1))
                # f2 of my rows, all heads -> allgather (concat on partitions)
                fall = p0.tile([2 * NHEADS, R], F32)
                nc.vector.tensor_copy(fall, bps)
                nc.sync.dma_start(
                    f_in.rearrange("one (h j) -> (one h) j", h=NHEADS),
                    fall[NHEADS:, :])
                nc.gpsimd.collective_compute(
                    "AllGather", OP.bypass, ins=[f_in.opt()], outs=[f_out.opt()],
                    replica_groups=rg)
                for h in range(NHEADS):
                    # row h -> partition 0 (SBUF->SBUF DMA), then broadcast
                    f1row = p0.tile([1, R], F32, name=f"f1row{h}", tag="f1row")
                    nc.sync.dma_start(f1row, fall[h:h + 1, :])
                    fb_ps = p0ps.tile([128, R], F32, name=f"fb_ps{h}", tag="fbps")
                    nc.tensor.matmul(fb_ps, lhsT=ones_row, rhs=f1row,
                                     start=True, stop=True)
                    nc.vector.tensor_copy(f1b[h], fb_ps)

            # =============== phase 1: Wh = x @ W[h] ===============
            with tc.tile_pool(name="p1s", bufs=2) as p1s, \
                 tc.tile_pool(name="p1ps", bufs=1, space="PSUM") as p1ps, \
                 tc.tile_pool(name="p1d", bufs=2) as p1d:
                xp1 = p0.tile([128, KT, R], F16)
                for q in range(4):
                    nc.vector.tensor_copy(xp1[:, q * 8:(q + 1) * 8, :],
                                          xres[:, q * 8:(q + 1) * 8, :].bitcast(F32))
                for h in range(NHEADS):
                    ps = [[p1ps.tile([128, 512], F32, name=f"ps_{h}_{i}_{oh}",
                                     tag=f"ps{i}{oh}") for oh in range(2)]
                          for i in range(IT)]
                    for kb in range(4):
                        wres = p1s.tile([128, 8, NHID], F16, tag="wres")
                        nc.sync.dma_start(
                            wres, W_d[h, :, kb * 8:(kb + 1) * 8, :])
                        for kk in range(8):
                            k = kb * 8 + kk
                            for i in range(IT):
                                for oh in range(2):
                                    nc.tensor.matmul(
                                        ps[i][oh],
                                        lhsT=xp1[:, k, i * 128:(i + 1) * 128],
                                        rhs=wres[:, kk, oh * 512:(oh + 1) * 512],
                                        start=(k == 0), stop=(k == KT - 1),
                                    )
                    for i in range(IT):
                        wh_sb = p1d.tile([128, NHID], F16, tag="wh_sb")
                        nc.vector.tensor_copy(wh_sb[:, :512], ps[i][0])
                        nc.vector.tensor_copy(wh_sb[:, 512:], ps[i][1])
                        nc.sync.dma_start(
                            ag1_in[h][:, i * NHID:(i + 1) * NHID], wh_sb)
                    # allgather this head's Wh while later heads compute
                    nc.gpsimd.collective_compute(
                        "AllGather", OP.bypass, ins=[ag1_in[h].opt()],
                        outs=[ag1_out[h].opt()], replica_groups=rg)

        # =============== phase 2: attention + aggregate, per head ===============
        p2c = ctx.enter_context(tc.tile_pool(name="p2c", bufs=1))
        xcatT = p2c.tile([128, KT, R], F16)
        wof = p2c.tile([128, KT, BIT], F32)
        nc.sync.dma_start(wof, wo_d)
        wob = p2c.tile([128, KT, BIT], F16)
        nc.vector.tensor_copy(wob, wof)

        pps = ctx.enter_context(tc.tile_pool(name="pps", bufs=1, space="PSUM"))
        p2s = ctx.enter_context(tc.tile_pool(name="p2s", bufs=2))
        p2w = ctx.enter_context(tc.tile_pool(name="p2w", bufs=2))
        p2p = ctx.enter_context(tc.tile_pool(name="p2p", bufs=4))

        for h in range(NHEADS):
            # f2 biases for this head: [128, 4, 8] -> (p, i, c)
            f2a = p2s.tile([128, IT, NC], F32, tag="f2a")
            for c in range(NC):
                nc.sync.dma_start(
                    f2a[:, :, c],
                    f_out[c:c + 1, h * R:(h + 1) * R].rearrange(
                        "one (b p) -> (one p) b", p=128))
            b1 = p2s.tile([128, IT, NC], F32, tag="b1")
            nc.vector.tensor_scalar_add(b1, f2a, BIAS_LN)
            b2 = p2s.tile([128, IT, NC], F32, tag="b2")
            nc.vector.tensor_scalar(b2, f2a, ALPHA, BIAS_LN, OP.mult, OP.add)

            rs_acc = p2s.tile([128, R], F32, tag="rs_acc")
            nc.vector.memset(rs_acc, 0.0)

            hps = [pps.tile([128, R], F32, name=f"hps{h}_{os}", tag=f"h{os}")
                   for os in range(8)]
            for c in range(NC):
                wht4 = p2w.tile([128, IT, NHID], F16, tag="wht", bufs=3)
                nc.sync.dma_start(
                    wht4, ag1_out[h][c * 128:(c + 1) * 128, :].rearrange(
                        "p (i o) -> p i o", i=IT))
                if h > 0:
                    _elu_tail(c)
                for i in range(IT):
                    jt = c * IT + i
                    e1 = p2p.tile([128, R], BF16, tag="e1")
                    nc.scalar.activation(e1, f1b[h], AF.Exp,
                                         bias=b1[:, i, c:c + 1], scale=1.0)
                    e2 = p2p.tile([128, R], BF16, tag="e2")
                    nc.scalar.activation(e2, f1b[h], AF.Exp,
                                         bias=b2[:, i, c:c + 1], scale=ALPHA)
                    nc.vector.tensor_tensor(e1, e1, e2, OP.max)
                    u = p2p.tile([128, R], BF16, tag="u")
                    nc.vector.tensor_tensor(u, e1, adjT[:, jt, :], OP.mult)
                    nc.gpsimd.tensor_tensor(rs_acc, rs_acc, u, OP.add)
                    for os in range(8):
                        nc.tensor.matmul(
                            hps[os], lhsT=wht4[:, i, os * 128:(os + 1) * 128],
                            rhs=u, start=(jt == 0), stop=(jt == JT - 1))

            # plain-drain bank 0 so the rowsum matmul can take its slot
            h0sb = p2s.tile([128, R], F32, tag="h0sb")
            nc.vector.tensor_copy(h0sb, hps[0])
            rs_ps = pps.tile([1, R], F32, name=f"rs_ps{h}", tag="h0")
            nc.tensor.matmul(rs_ps, lhsT=ones_col, rhs=rs_acc, start=True, stop=True)
            recip = p2s.tile([1, R], F32, tag="recip")
            nc.vector.reciprocal(recip, rs_ps)
            bc_ps = pps.tile([128, R], F32, name=f"bc_ps{h}", tag="h0")
            nc.tensor.matmul(bc_ps, lhsT=ones_row, rhs=recip, start=True, stop=True)
            rb = p2s.tile([128, R], F32, tag="rb")
            nc.vector.tensor_copy(rb, bc_ps)

            hstage = p2s.tile([128, 8, R], F16, name=f"hstage{h}", tag="hstage",
                              bufs=1)
            for os in range(8):
                nc.vector.tensor_tensor(hstage[:, os, :],
                                        h0sb if os == 0 else hps[os], rb, OP.mult)


            def _elu_tail(os, h=h, hstage=hstage):
                mn = p2w.tile([128, R], F16, tag="u2f")
                nc.vector.tensor_scalar_min(mn, hstage[:, os, :], 0.0)
                ex = p2w.tile([128, R], F16, tag="ex")
                nc.scalar.activation(ex, mn, AF.Exp)
                nc.vector.scalar_tensor_tensor(
                    out=xcatT[:, h * 8 + os, :], in0=ex, scalar=-1.0,
                    in1=hstage[:, os, :], op0=OP.add, op1=OP.max)

        for os in range(8):
            _elu_tail(os)

        # =============== phase 3: Wh2 = x_cat @ W_out; g1/g2 ===============
        wh2T_ps = pps.tile([BIT, R], F32, tag="h2")
        for k in range(KT):
            nc.tensor.matmul(wh2T_ps, lhsT=wob[:, k, :], rhs=xcatT[:, k, :],
                             start=(k == 0), stop=(k == KT - 1))
        wh2T = p2c.tile([BIT, R], F32)
        nc.vector.tensor_copy(wh2T, wh2T_ps)
        g1T_ps = pps.tile([1, R], F32, tag="h3")
        nc.tensor.matmul(g1T_ps, lhsT=a1o_col, rhs=wh2T, start=True, stop=True)
        g1T = p2c.tile([1, R], F32)
        nc.vector.tensor_copy(g1T, g1T_ps)

        for i in range(IT):
            tp_ps = pps.tile([128, BIT], F32, name=f"w2t{i}", tag="h4")
            nc.tensor.transpose(tp_ps, wh2T[:, i * 128:(i + 1) * 128],
                                ident[:BIT, :BIT])
            wh2n = p2w.tile([128, BIT], F32, tag="wh2n")
            nc.vector.tensor_copy(wh2n, tp_ps)
            g2c = p2w.tile([128, 1], F32, tag="g2c")
            scratch2 = p2w.tile([128, BIT], F32, tag="scratch2")
            nc.vector.scalar_tensor_tensor(
                out=scratch2, in0=wh2n, scalar=0.0, in1=a2o_b,
                op0=OP.bypass, op1=OP.mult, accum_out=g2c)
            base = i * (BIT + 1)
            nc.sync.dma_start(ag2_in[:, base:base + BIT], wh2n)
            nc.sync.dma_start(ag2_in[:, base + BIT:base + BIT + 1], g2c)

        nc.gpsimd.collective_compute(
            "AllGather", OP.bypass, ins=[ag2_in.opt()], outs=[ag2_out.opt()],
            replica_groups=rg)

        # =============== phase 4: output attention ===============
        g1b_ps = pps.tile([128, R], F32, tag="h5")
        nc.tensor.matmul(g1b_ps, lhsT=ones_row, rhs=g1T, start=True, stop=True)
        g1b = p2c.tile([128, R], F32)
        nc.vector.tensor_copy(g1b, g1b_ps)

        # 4-way accumulator tree keeps the serial GpSimd chain short
        rs2_acc = [p2s.tile([128, R], F32, name=f"rs2_{a}", tag=f"rs2_{a}")
                   for a in range(4)]
        for a in range(4):
            nc.vector.memset(rs2_acc[a], 0.0)
        ht2_ps = pps.tile([BIT, R], F32, tag="h6")
        for c in range(NC):
            w2t4 = p2w.tile([128, IT, BIT + 1], F32, tag="w2t4")
            nc.sync.dma_start(
                w2t4, ag2_out[c * 128:(c + 1) * 128, :].rearrange(
                    "p (i z) -> p i z", i=IT))
            g2s4 = p2w.tile([128, IT], F32, tag="g2s4")
            nc.vector.tensor_scalar_mul(g2s4, w2t4[:, :, BIT], ALPHA)
            w2b = p2w.tile([128, IT, BIT], F16, tag="w2b")
            nc.vector.tensor_copy(w2b, w2t4[:, :, :BIT])
            for i in range(IT):
                jt = c * IT + i
                e1 = p2p.tile([128, R], BF16, tag="e1")
                nc.scalar.activation(e1, g1b, AF.Exp,
                                     bias=w2t4[:, i, BIT:BIT + 1], scale=1.0)
                e2 = p2p.tile([128, R], BF16, tag="e2")
                nc.scalar.activation(e2, g1b, AF.Exp,
                                     bias=g2s4[:, i:i + 1], scale=ALPHA)
                nc.vector.tensor_tensor(e1, e1, e2, OP.max)
                u2 = p2p.tile([128, R], BF16, tag="u")
                nc.vector.tensor_tensor(u2, e1, adjT[:, jt, :], OP.mult)
                eng = nc.gpsimd if jt % 4 < 2 else nc.vector
                eng.tensor_tensor(rs2_acc[jt % 4], rs2_acc[jt % 4], u2, OP.add)
                nc.tensor.matmul(ht2_ps, lhsT=w2b[:, i, :], rhs=u2,
                                 start=(jt == 0), stop=(jt == JT - 1))

        nc.vector.tensor_tensor(rs2_acc[0], rs2_acc[0], rs2_acc[1], OP.add)
        nc.vector.tensor_tensor(rs2_acc[2], rs2_acc[2], rs2_acc[3], OP.add)
        nc.vector.tensor_tensor(rs2_acc[0], rs2_acc[0], rs2_acc[2], OP.add)
        rs2_ps = pps.tile([1, R], F32, tag="h7")
        nc.tensor.matmul(rs2_ps, lhsT=ones_col, rhs=rs2_acc[0], start=True, stop=True)
        recip2 = p2c.tile([1, R], F32)
        nc.vector.reciprocal(recip2, rs2_ps)
        bc2_ps = pps.tile([128, R], F32, tag="h0")
        nc.tensor.matmul(bc2_ps, lhsT=ones_row, rhs=recip2, start=True, stop=True)
        rb2 = p2c.tile([128, R], F32)
        nc.vector.tensor_copy(rb2, bc2_ps)

        ot = p2c.tile([BIT, R], F32)
        nc.vector.tensor_tensor(ot, ht2_ps, rb2[:BIT, :], OP.mult)
        outT = p2c.tile([BIT, R], F32)
        nc.scalar.activation(outT, ot, AF.Tanh)
        for i in range(IT):
            tp_ps = pps.tile([128, BIT], F32, name=f"ot{i}", tag="h1")
            nc.tensor.transpose(tp_ps, outT[:, i * 128:(i + 1) * 128],
                                ident[:BIT, :BIT])
            ob = p2w.tile([128, BIT], F32, tag="ob")
            nc.vector.tensor_copy(ob, tp_ps)
            nc.sync.dma_start(out_d[i * 128:(i + 1) * 128, :], ob)

    _split_excess_waits(nc, max_waits=1)
    return nc


_CACHED = None


def _get_program():
    global _CACHED
    if _CACHED is None:
        _CACHED = build_program()
    return _CACHED


def _interleave(a, kt):
    """[kt*128, free...] -> [128, kt, free...] partition-major."""
    return np.ascontiguousarray(
        a.reshape(kt, 128, *a.shape[1:]).transpose(1, 0, *range(2, a.ndim + 1)))


def make_in_maps(x, adj, W, a1, a2, W_out, a1_out, a2_out):
    import ml_dtypes
    xT = np.ascontiguousarray(x.T)
    adjT_bf = adj.T.astype(ml_dtypes.bfloat16)
    # B = [W[h] @ a1[h] (4 cols) | W[h] @ a2[h] (4 cols)]  (fp32 logit vecs)
    B = np.concatenate(
        [np.stack([W[h] @ a1[h] for h in range(NHEADS)], axis=1),
         np.stack([W[h] @ a2[h] for h in range(NHEADS)], axis=1)],
        axis=1).astype(np.float32)
    # W interleaved: [h, 128, KT, NHID]
    W_sh = np.ascontiguousarray(
        W.reshape(NHEADS, KT, 128, NHID).transpose(0, 2, 1, 3)
).astype(np.float16)
    B_sh = _interleave(B, KT)
    Wo_sh = _interleave(W_out, KT)
    in_maps = []
    for d in range(NC):
        cols = slice(d * R, (d + 1) * R)
        in_maps.append({
            "x_sh": _interleave(np.ascontiguousarray(xT[:, cols]), KT),
            "W_sh": W_sh,
            "B_sh": B_sh,
            "adj_sh": _interleave(np.ascontiguousarray(adjT_bf[:, cols]), JT),
            "Wo_sh": Wo_sh,
            "a1_out": a1_out, "a2_out": a2_out,
        })
    return in_maps


def kernel(x, adj, W, a1, a2, W_out, a1_out, a2_out, _trace=False):
    nc = _get_program()
    in_maps = make_in_maps(np.asarray(x, np.float32), np.asarray(adj, np.float32),
                           np.asarray(W, np.float32), np.asarray(a1, np.float32),
                           np.asarray(a2, np.float32), np.asarray(W_out, np.float32),
                           np.asarray(a1_out, np.float32),
                           np.asarray(a2_out, np.float32))
    res = bass_utils.run_bass_kernel_spmd(
        nc, in_maps, core_ids=list(range(NC)), trace=_trace)
    out = np.concatenate([res.results[d]["out_rows"] for d in range(NC)], axis=0)
    if _trace:
        kernel.last_exec_time_ns = res.exec_time_ns
        kernel.last_results = res
    return out


# revision 33
# speedup vs baseline: 1.0247x; 1.0143x over previous
"""Trainium2 Bass kernel for nn_GATNet_IMG (dense 2-layer GAT, N=4096).

Sharding: 1D row-parallel over the node dim across 8 NeuronCores.
Each core computes Wh for its 512 rows (all 4 heads), AllGathers Wh
per head (overlapped with the next head's matmuls), then computes its
[512, 4096] attention block per head with a fused masked softmax (no
NxN matrix ever hits HBM), aggregates h^T = Wh^T @ u on TensorE, and
repeats the same pattern for the output attention layer.

Key tricks:
  - exp(leaky_relu(s)) == max(exp(s), exp(0.2*s))  (exact, all s)
  - attention logit halves f1/f2 come from one tiny [8]-wide matmul
    x @ B where B = [W[h]@a1[h] | W[h]@a2[h]] is host-precomputed, so
    logit precision is independent of the big-GEMM compute dtype
  - unnormalized attention + row-sum via ones-matmul; normalization is
    a post-scale of h^T (free-axis scale via a broadcast matmul)
  - elu(x) == max(exp(min(x, 0)) - 1, x)           (exact)
  - ln(2^-30) folded into the exp bias keeps row sums in DVE
    reciprocal range
  - host-side sharding pre-transposes x/adj AND interleaves all heavy
    streams into [128, ktiles, free] partition-major layouts so every
    DMA moves 8-64KB contiguous per partition (packet-rate limit)
"""

import math
from contextlib import ExitStack

import numpy as np

import concourse.bass as bass
import concourse.mybir as mybir
import concourse.tile as tile
from concourse import bass_utils
from concourse.masks import make_identity

F32 = mybir.dt.float32
F32R = mybir.dt.float32r
F16 = mybir.dt.float16
BF16 = mybir.dt.bfloat16
AF = mybir.ActivationFunctionType
OP = mybir.AluOpType

N = 4096
NFEAT = 4096
NHID = 1024
NHEADS = 4
BIT = 64
NC = 8
R = N // NC          # 512 attention rows per core
KT = NFEAT // 128    # 32 k tiles
JT = N // 128        # 32 node-column tiles
IT = R // 128        # 4 row tiles per core
ALPHA = 0.2
BIAS_LN = -30.0 * math.log(2.0)   # ln(2^-30) folded into exp bias (layer 1)


def _split_excess_waits(nc, max_waits=1):
    """walrus codegen rejects >max_waits sync-wait commands per instruction;
    push excess waits onto preceding same-engine NoOps."""
    n_fixed = 0
    for f in nc.m.functions:
        for b in f.blocks:
            new_insts = []
            changed = False
            for inst in b.instructions:
                si = getattr(inst, "sync_info", None)
                if si is not None and si.on_wait and len(si.on_wait) > max_waits:
                    waits = list(si.on_wait)
                    excess, keep = waits[:-max_waits], waits[-max_waits:]
                    for ci in range(0, len(excess), max_waits):
                        nop = mybir.InstNoOp(
                            name=f"{inst.name}-ws{ci}",
                            sync_info=mybir.SyncInfo(
                                on_wait=excess[ci:ci + max_waits], on_update=[]
                            ),
                            bass_nofuse=True,
                            engine=inst.engine,
                        )
                        new_insts.append(nop)
                    inst.sync_info = mybir.SyncInfo(
                        on_wait=keep, on_update=list(si.on_update or [])
                    )
                    n_fixed += 1
                    changed = True
                new_insts.append(inst)
            if changed:
                insts = b.instructions
                try:
                    b.instructions = new_insts
                except Exception:
                    while len(insts):
                        insts.pop()
                    for i in new_insts:
                        insts.append(i)
    return n_fixed


def build_program():
    nc = bass.Bass("TRN2", target_bir_lowering=False, debug=False, num_devices=NC)

    # host-interleaved inputs: [128 partitions, ktiles, free]
    x_d = nc.dram_tensor("x_sh", [128, KT, R], F32, kind="ExternalInput").ap()
    W_d = nc.dram_tensor("W_sh", [NHEADS, 128, KT, NHID], F16,
                         kind="ExternalInput").ap()
    B_d = nc.dram_tensor("B_sh", [128, KT, 2 * NHEADS], F32,
                         kind="ExternalInput").ap()
    adj_d = nc.dram_tensor("adj_sh", [128, JT, R], BF16, kind="ExternalInput").ap()
    wo_d = nc.dram_tensor("Wo_sh", [128, KT, BIT], F32, kind="ExternalInput").ap()
    a1o_d = nc.dram_tensor("a1_out", [BIT], F32, kind="ExternalInput").ap()
    a2o_d = nc.dram_tensor("a2_out", [BIT], F32, kind="ExternalInput").ap()
    out_d = nc.dram_tensor("out_rows", [R, BIT], F32, kind="ExternalOutput").ap()

    # collective bounce buffers; Wh per head so each head's allgather overlaps
    # the next head's phase-1 compute. Layout: [128, i*NHID] per rank so the
    # gathered output reads back with 8KB/partition lines.
    ag1_in = [nc.dram_tensor(f"ag1_in{h}", [128, IT * NHID], F16).ap()
              for h in range(NHEADS)]
    ag1_out = [nc.dram_tensor(f"ag1_out{h}", [NC * 128, IT * NHID], F16,
                              addr_space="Shared").ap() for h in range(NHEADS)]
    f_in = nc.dram_tensor("f_in", [1, NHEADS * R], F32).ap()
    f_out = nc.dram_tensor("f_out", [NC, NHEADS * R], F32, addr_space="Shared").ap()
    ag2_in = nc.dram_tensor("ag2_in", [128, IT * (BIT + 1)], F32).ap()
    ag2_out = nc.dram_tensor("ag2_out", [NC * 128, IT * (BIT + 1)], F32,
                             addr_space="Shared").ap()

    rg = [list(range(NC))]

    with tile.TileContext(nc) as tc, ExitStack() as ctx:
        cp = ctx.enter_context(tc.tile_pool(name="const", bufs=1))
        ident = cp.tile([128, 128], F32)
        make_identity(nc, ident)
        ones_col = cp.tile([128, 1], F32)
        nc.vector.memset(ones_col, 1.0)
        ones_row = cp.tile([1, 128], F32)
        nc.vector.memset(ones_row, 1.0)
        a1o_col = cp.tile([BIT, 1], F32)
        nc.sync.dma_start(a1o_col, a1o_d.rearrange("(b one) -> b one", one=1))
        a2o_b = cp.tile([128, BIT], F32)
        nc.sync.dma_start(
            a2o_b, a2o_d.rearrange("(one b) -> one b", one=1).to_broadcast([128, BIT]))
        # adjacency mask, resident for both attention layers (one big DMA)
        adjT = cp.tile([128, JT, R], BF16)
        nc.sync.dma_start(adjT, adj_d)
        # f1 broadcast tiles [128, R] per head
        f1b = [cp.tile([128, R], F32, name=f"f1b_{h}") for h in range(NHEADS)]

        # =============== phase 0: f-logit halves via x @ B ===============
        with tc.tile_pool(name="p0", bufs=1) as p0:
            with tc.tile_pool(name="p0ps", bufs=1, space="PSUM") as p0ps:
                bres = p0.tile([128, KT, 2 * NHEADS], F32R)
                nc.sync.dma_start(bres, B_d.bitcast(F32R))
                xres = p0.tile([128, KT, R], F32R)
                for q in range(4):
                    nc.sync.dma_start(xres[:, q * 8:(q + 1) * 8, :],
                                      x_d[:, q * 8:(q + 1) * 8, :].bitcast(F32R))
                bps = p0ps.tile([2 * NHEADS, R], F32)
                for k in range(KT):
                    nc.tensor.matmul(bps, lhsT=bres[:, k, :], rhs=xres[:, k, :],
                                     start=(k == 0), stop=(k == KT - 1))
                # f2 of my rows, all heads -> allgather (concat on partitions)
                fall = p0.tile([2 * NHEADS, R], F32)
                nc.vector.tensor_copy(fall, bps)
                nc.sync.dma_start(
                    f_in.rearrange("one (h j) -> (one h) j", h=NHEADS),
                    fall[NHEADS:, :])
                nc.gpsimd.collective_compute(
                    "AllGather", OP.bypass, ins=[f_in.opt()], outs=[f_out.opt()],
                    replica_groups=rg)
                for h in range(NHEADS):
                    # row h -> partition 0 (SBUF->SBUF DMA), then broadcast
                    f1row = p0.tile([1, R], F32, name=f"f1row{h}", tag="f1row")
                    nc.sync.dma_start(f1row, fall[h:h + 1, :])
                    fb_ps = p0ps.tile([128, R], F32, name=f"fb_ps{h}", tag="fbps")
                    nc.tensor.matmul(fb_ps, lhsT=ones_row, rhs=f1row,
                                     start=True, stop=True)
                    nc.vector.tensor_copy(f1b[h], fb_ps)

            # =============== phase 1: Wh = x @ W[h] ===============
            with tc.tile_pool(name="p1s", bufs=3) as p1s, \
                 tc.tile_pool(name="p1ps", bufs=1, space="PSUM") as p1ps, \
                 tc.tile_pool(name="p1d", bufs=2) as p1d:
                xp1 = p0.tile([128, KT, R], F16)
                for q in range(4):
                    nc.vector.tensor_copy(xp1[:, q * 8:(q + 1) * 8, :],
                                          xres[:, q * 8:(q + 1) * 8, :].bitcast(F32))
                for h in range(NHEADS):
                    ps = [[p1ps.tile([128, 512], F32, name=f"ps_{h}_{i}_{oh}",
                                     tag=f"ps{i}{oh}") for oh in range(2)]
                          for i in range(IT)]
                    for kb in range(4):
                        wres = p1s.tile([128, 8, NHID], F16, tag="wres")
                        nc.sync.dma_start(
                            wres, W_d[h, :, kb * 8:(kb + 1) * 8, :])
                        for kk in range(8):
                            k = kb * 8 + kk
                            for i in range(IT):
                                for oh in range(2):
                                    nc.tensor.matmul(
                                        ps[i][oh],
                                        lhsT=xp1[:, k, i * 128:(i + 1) * 128],
                                        rhs=wres[:, kk, oh * 512:(oh + 1) * 512],
                                        start=(k == 0), stop=(k == KT - 1),
                                    )
                    for i in range(IT):
                        wh_sb = p1d.tile([128, NHID], F16, tag="wh_sb")
                        nc.vector.tensor_copy(wh_sb[:, :512], ps[i][0])
                        nc.vector.tensor_copy(wh_sb[:, 512:], ps[i][1])
                        nc.sync.dma_start(
                            ag1_in[h][:, i * NHID:(i + 1) * NHID], wh_sb)
                    # allgather this head's Wh while later heads compute
                    nc.gpsimd.collective_compute(
                        "AllGather", OP.bypass, ins=[ag1_in[h].opt()],
                        outs=[ag1_out[h].opt()], replica_groups=rg)

        # =============== phase 2: attention + aggregate, per head ===============
        p2c = ctx.enter_context(tc.tile_pool(name="p2c", bufs=1))
        xcatT = p2c.tile([128, KT, R], F16)
        wof = p2c.tile([128, KT, BIT], F32)
        nc.sync.dma_start(wof, wo_d)
        wob = p2c.tile([128, KT, BIT], F16)
        nc.vector.tensor_copy(wob, wof)

        pps = ctx.enter_context(tc.tile_pool(name="pps", bufs=1, space="PSUM"))
        p2s = ctx.enter_context(tc.tile_pool(name="p2s", bufs=2))
        p2w = ctx.enter_context(tc.tile_pool(name="p2w", bufs=2))
        p2p = ctx.enter_context(tc.tile_pool(name="p2p", bufs=4))

        for h in range(NHEADS):
            # f2 biases for this head: [128, 4, 8] -> (p, i, c)
            f2a = p2s.tile([128, IT, NC], F32, tag="f2a")
            for c in range(NC):
                nc.sync.dma_start(
                    f2a[:, :, c],
                    f_out[c:c + 1, h * R:(h + 1) * R].rearrange(
                        "one (b p) -> (one p) b", p=128))
            b1 = p2s.tile([128, IT, NC], F32, tag="b1")
            nc.vector.tensor_scalar_add(b1, f2a, BIAS_LN)
            b2 = p2s.tile([128, IT, NC], F32, tag="b2")
            nc.vector.tensor_scalar(b2, f2a, ALPHA, BIAS_LN, OP.mult, OP.add)

            rs_acc = p2s.tile([128, R], F32, tag="rs_acc")
            nc.vector.memset(rs_acc, 0.0)

            hps = [pps.tile([128, R], F32, name=f"hps{h}_{os}", tag=f"h{os}")
                   for os in range(8)]
            for c in range(NC):
                wht4 = p2w.tile([128, IT, NHID], F16, tag="wht", bufs=3)
                nc.sync.dma_start(
                    wht4, ag1_out[h][c * 128:(c + 1) * 128, :].rearrange(
                        "p (i o) -> p i o", i=IT))
                if h > 0:
                    _elu_tail(c)
                for i in range(IT):
                    jt = c * IT + i
                    e1 = p2p.tile([128, R], BF16, tag="e1")
                    nc.scalar.activation(e1, f1b[h], AF.Exp,
                                         bias=b1[:, i, c:c + 1], scale=1.0)
                    e2 = p2p.tile([128, R], BF16, tag="e2")
                    nc.scalar.activation(e2, f1b[h], AF.Exp,
                                         bias=b2[:, i, c:c + 1], scale=ALPHA)
                    nc.vector.tensor_tensor(e1, e1, e2, OP.max)
                    u = p2p.tile([128, R], BF16, tag="u")
                    nc.vector.tensor_tensor(u, e1, adjT[:, jt, :], OP.mult)
                    nc.gpsimd.tensor_tensor(rs_acc, rs_acc, u, OP.add)
                    for os in range(8):
                        nc.tensor.matmul(
                            hps[os], lhsT=wht4[:, i, os * 128:(os + 1) * 128],
                            rhs=u, start=(jt == 0), stop=(jt == JT - 1))

            # plain-drain bank 0 so the rowsum matmul can take its slot
            h0sb = p2s.tile([128, R], F32, tag="h0sb")
            nc.vector.tensor_copy(h0sb, hps[0])
            rs_ps = pps.tile([1, R], F32, name=f"rs_ps{h}", tag="h0")
            nc.tensor.matmul(rs_ps, lhsT=ones_col, rhs=rs_acc, start=True, stop=True)
            recip = p2s.tile([1, R], F32, tag="recip")
            nc.vector.reciprocal(recip, rs_ps)
            bc_ps = pps.tile([128, R], F32, name=f"bc_ps{h}", tag="h0")
            nc.tensor.matmul(bc_ps, lhsT=ones_row, rhs=recip, start=True, stop=True)
            rb = p2s.tile([128, R], F32, tag="rb")
            nc.vector.tensor_copy(rb, bc_ps)

            hstage = p2s.tile([128, 8, R], F16, name=f"hstage{h}", tag="hstage",
                              bufs=1)
            for os in range(8):
                nc.vector.tensor_tensor(hstage[:, os, :],
                                        h0sb if os == 0 else hps[os], rb, OP.mult)


            def _elu_tail(os, h=h, hstage=hstage):
                mn = p2w.tile([128, R], F16, tag="u2f")
                nc.vector.tensor_scalar_min(mn, hstage[:, os, :], 0.0)
                ex = p2w.tile([128, R], F16, tag="ex")
                nc.scalar.activation(ex, mn, AF.Exp)
                nc.vector.scalar_tensor_tensor(
                    out=xcatT[:, h * 8 + os, :], in0=ex, scalar=-1.0,
                    in1=hstage[:, os, :], op0=OP.add, op1=OP.max)

        for os in range(8):
            _elu_tail(os)

        # =============== phase 3: Wh2 = x_cat @ W_out; g1/g2 ===============
        wh2T_ps = pps.tile([BIT, R], F32, tag="h2")
        for k in range(KT):
            nc.tensor.matmul(wh2T_ps, lhsT=wob[:, k, :], rhs=xcatT[:, k, :],
                             start=(k == 0), stop=(k == KT - 1))
        wh2T = p2c.tile([BIT, R], F32)
        nc.vector.tensor_copy(wh2T, wh2T_ps)
        g1T_ps = pps.tile([1, R], F32, tag="h3")
        nc.tensor.matmul(g1T_ps, lhsT=a1o_col, rhs=wh2T, start=True, stop=True)
        g1T = p2c.tile([1, R], F32)
        nc.vector.tensor_copy(g1T, g1T_ps)

        for i in range(IT):
            tp_ps = pps.tile([128, BIT], F32, name=f"w2t{i}", tag="h4")
            nc.tensor.transpose(tp_ps, wh2T[:, i * 128:(i + 1) * 128],
                                ident[:BIT, :BIT])
            wh2n = p2w.tile([128, BIT], F32, tag="wh2n")
            nc.vector.tensor_copy(wh2n, tp_ps)
            g2c = p2w.tile([128, 1], F32, tag="g2c")
            scratch2 = p2w.tile([128, BIT], F32, tag="scratch2")
            nc.vector.scalar_tensor_tensor(
                out=scratch2, in0=wh2n, scalar=0.0, in1=a2o_b,
                op0=OP.bypass, op1=OP.mult, accum_out=g2c)
            base = i * (BIT + 1)
            nc.sync.dma_start(ag2_in[:, base:base + BIT], wh2n)
            nc.sync.dma_start(ag2_in[:, base + BIT:base + BIT + 1], g2c)

        nc.gpsimd.collective_compute(
            "AllGather", OP.bypass, ins=[ag2_in.opt()], outs=[ag2_out.opt()],
            replica_groups=rg)

        # =============== phase 4: output attention ===============
        g1b_ps = pps.tile([128, R], F32, tag="h5")
        nc.tensor.matmul(g1b_ps, lhsT=ones_row, rhs=g1T, start=True, stop=True)
        g1b = p2c.tile([128, R], F32)
        nc.vector.tensor_copy(g1b, g1b_ps)

        # 4-way accumulator tree keeps the serial GpSimd chain short
        rs2_acc = [p2s.tile([128, R], F32, name=f"rs2_{a}", tag=f"rs2_{a}")
                   for a in range(4)]
        for a in range(4):
            nc.vector.memset(rs2_acc[a], 0.0)
        ht2_ps = pps.tile([BIT, R], F32, tag="h6")
        for c in range(NC):
            w2t4 = p2w.tile([128, IT, BIT + 1], F32, tag="w2t4")
            nc.sync.dma_start(
                w2t4, ag2_out[c * 128:(c + 1) * 128, :].rearrange(
                    "p (i z) -> p i z", i=IT))
            g2s4 = p2w.tile([128, IT], F32, tag="g2s4")
            nc.vector.tensor_scalar_mul(g2s4, w2t4[:, :, BIT], ALPHA)
            w2b = p2w.tile([128, IT, BIT], F16, tag="w2b")
            nc.vector.tensor_copy(w2b, w2t4[:, :, :BIT])
            for i in range(IT):
                jt = c * IT + i
                e1 = p2p.tile([128, R], BF16, tag="e1")
                nc.scalar.activation(e1, g1b, AF.Exp,
                                     bias=w2t4[:, i, BIT:BIT + 1], scale=1.0)
                e2 = p2p.tile([128, R], BF16, tag="e2")
                nc.scalar.activation(e2, g1b, AF.Exp,
                                     bias=g2s4[:, i:i + 1], scale=ALPHA)
                nc.vector.tensor_tensor(e1, e1, e2, OP.max)
                u2 = p2p.tile([128, R], BF16, tag="u")
                nc.vector.tensor_tensor(u2, e1, adjT[:, jt, :], OP.mult)
                eng = nc.gpsimd if jt % 4 < 2 else nc.vector
                eng.tensor_tensor(rs2_acc[jt % 4], rs2_acc[jt % 4], u2, OP.add)
                nc.tensor.matmul(ht2_ps, lhsT=w2b[:, i, :], rhs=u2,
                                 start=(jt == 0), stop=(jt == JT - 1))

        nc.vector.tensor_tensor(rs2_acc[0], rs2_acc[0], rs2_acc[1], OP.add)
        nc.vector.tensor_tensor(rs2_acc[2], rs2_acc[2], rs2_acc[3], OP.add)
        nc.vector.tensor_tensor(rs2_acc[0], rs2_acc[0], rs2_acc[2], OP.add)
        rs2_ps = pps.tile([1, R], F32, tag="h7")
        nc.tensor.matmul(rs2_ps, lhsT=ones_col, rhs=rs2_acc[0], start=True, stop=True)
        recip2 = p2c.tile([1, R], F32)
        nc.vector.reciprocal(recip2, rs2_ps)
        bc2_ps = pps.tile([128, R], F32, tag="h0")
        nc.tensor.matmul(bc2_ps, lhsT=ones_row, rhs=recip2, start=True, stop=True)
        rb2 = p2c.tile([128, R], F32)
        nc.vector.tensor_copy(rb2, bc2_ps)

        ot = p2c.tile([BIT, R], F32)
        nc.vector.tensor_tensor(ot, ht2_ps, rb2[:BIT, :], OP.mult)
        outT = p2c.tile([BIT, R], F32)
        nc.scalar.activation(outT, ot, AF.Tanh)
        for i in range(IT):
            tp_ps = pps.tile([128, BIT], F32, name=f"ot{i}", tag="h1")
            nc.tensor.transpose(tp_ps, outT[:, i * 128:(i + 1) * 128],
                                ident[:BIT, :BIT])
            ob = p2w.tile([128, BIT], F32, tag="ob")
            nc.vector.tensor_copy(ob, tp_ps)
            nc.sync.dma_start(out_d[i * 128:(i + 1) * 128, :], ob)

    _split_excess_waits(nc, max_waits=1)
    return nc


_CACHED = None


def _get_program():
    global _CACHED
    if _CACHED is None:
        _CACHED = build_program()
    return _CACHED


def _interleave(a, kt):
    """[kt*128, free...] -> [128, kt, free...] partition-major."""
    return np.ascontiguousarray(
        a.reshape(kt, 128, *a.shape[1:]).transpose(1, 0, *range(2, a.ndim + 1)))


def make_in_maps(x, adj, W, a1, a2, W_out, a1_out, a2_out):
    import ml_dtypes
    xT = np.ascontiguousarray(x.T)
    adjT_bf = adj.T.astype(ml_dtypes.bfloat16)
    # B = [W[h] @ a1[h] (4 cols) | W[h] @ a2[h] (4 cols)]  (fp32 logit vecs)
    B = np.concatenate(
        [np.stack([W[h] @ a1[h] for h in range(NHEADS)], axis=1),
         np.stack([W[h] @ a2[h] for h in range(NHEADS)], axis=1)],
        axis=1).astype(np.float32)
    # W interleaved: [h, 128, KT, NHID]
    W_sh = np.ascontiguousarray(
        W.reshape(NHEADS, KT, 128, NHID).transpose(0, 2, 1, 3)
).astype(np.float16)
    B_sh = _interleave(B, KT)
    Wo_sh = _interleave(W_out, KT)
    in_maps = []
    for d in range(NC):
        cols = slice(d * R, (d + 1) * R)
        in_maps.append({
            "x_sh": _interleave(np.ascontiguousarray(xT[:, cols]), KT),
            "W_sh": W_sh,
            "B_sh": B_sh,
            "adj_sh": _interleave(np.ascontiguousarray(adjT_bf[:, cols]), JT),
            "Wo_sh": Wo_sh,
            "a1_out": a1_out, "a2_out": a2_out,
        })
    return in_maps


def kernel(x, adj, W, a1, a2, W_out, a1_out, a2_out, _trace=False):
    nc = _get_program()
    in_maps = make_in_maps(np.asarray(x, np.float32), np.asarray(adj, np.float32),
                           np.asarray(W, np.float32), np.asarray(a1, np.float32),
                           np.asarray(a2, np.float32), np.asarray(W_out, np.float32),
                           np.asarray(a1_out, np.float32),
                           np.asarray(a2_out, np.float32))
    res = bass_utils.run_bass_kernel_spmd(
        nc, in_maps, core_ids=list(range(NC)), trace=_trace)
    out = np.concatenate([res.results[d]["out_rows"] for d in range(NC)], axis=0)
    if _trace:
        kernel.last_exec_time_ns = res.exec_time_ns
        kernel.last_results = res
    return out


# revision 34
# speedup vs baseline: 1.0338x; 1.0089x over previous
"""Trainium2 Bass kernel for nn_GATNet_IMG (dense 2-layer GAT, N=4096).

Sharding: 1D row-parallel over the node dim across 8 NeuronCores.
Each core computes Wh for its 512 rows (all 4 heads), AllGathers Wh
per head (overlapped with the next head's matmuls), then computes its
[512, 4096] attention block per head with a fused masked softmax (no
NxN matrix ever hits HBM), aggregates h^T = Wh^T @ u on TensorE, and
repeats the same pattern for the output attention layer.

Key tricks:
  - exp(leaky_relu(s)) == max(exp(s), exp(0.2*s))  (exact, all s)
  - attention logit halves f1/f2 come from one tiny [8]-wide matmul
    x @ B where B = [W[h]@a1[h] | W[h]@a2[h]] is host-precomputed, so
    logit precision is independent of the big-GEMM compute dtype
  - unnormalized attention + row-sum via ones-matmul; normalization is
    a post-scale of h^T (free-axis scale via a broadcast matmul)
  - elu(x) == max(exp(min(x, 0)) - 1, x)           (exact)
  - ln(2^-30) folded into the exp bias keeps row sums in DVE
    reciprocal range
  - host-side sharding pre-transposes x/adj AND interleaves all heavy
    streams into [128, ktiles, free] partition-major layouts so every
    DMA moves 8-64KB contiguous per partition (packet-rate limit)
"""

import math
from contextlib import ExitStack

import numpy as np

import concourse.bass as bass
import concourse.mybir as mybir
import concourse.tile as tile
from concourse import bass_utils
from concourse.masks import make_identity

F32 = mybir.dt.float32
F32R = mybir.dt.float32r
F16 = mybir.dt.float16
BF16 = mybir.dt.bfloat16
AF = mybir.ActivationFunctionType
OP = mybir.AluOpType

N = 4096
NFEAT = 4096
NHID = 1024
NHEADS = 4
BIT = 64
NC = 8
R = N // NC          # 512 attention rows per core
KT = NFEAT // 128    # 32 k tiles
JT = N // 128        # 32 node-column tiles
IT = R // 128        # 4 row tiles per core
ALPHA = 0.2
BIAS_LN = -30.0 * math.log(2.0)   # ln(2^-30) folded into exp bias (layer 1)


def _split_excess_waits(nc, max_waits=1):
    """walrus codegen rejects >max_waits sync-wait commands per instruction;
    push excess waits onto preceding same-engine NoOps."""
    n_fixed = 0
    for f in nc.m.functions:
        for b in f.blocks:
            new_insts = []
            changed = False
            for inst in b.instructions:
                si = getattr(inst, "sync_info", None)
                if si is not None and si.on_wait and len(si.on_wait) > max_waits:
                    waits = list(si.on_wait)
                    excess, keep = waits[:-max_waits], waits[-max_waits:]
                    for ci in range(0, len(excess), max_waits):
                        nop = mybir.InstNoOp(
                            name=f"{inst.name}-ws{ci}",
                            sync_info=mybir.SyncInfo(
                                on_wait=excess[ci:ci + max_waits], on_update=[]
                            ),
                            bass_nofuse=True,
                            engine=inst.engine,
                        )
                        new_insts.append(nop)
                    inst.sync_info = mybir.SyncInfo(
                        on_wait=keep, on_update=list(si.on_update or [])
                    )
                    n_fixed += 1
                    changed = True
                new_insts.append(inst)
            if changed:
                insts = b.instructions
                try:
                    b.instructions = new_insts
                except Exception:
                    while len(insts):
                        insts.pop()
                    for i in new_insts:
                        insts.append(i)
    return n_fixed


def build_program():
    nc = bass.Bass("TRN2", target_bir_lowering=False, debug=False, num_devices=NC)

    # host-interleaved inputs: [128 partitions, ktiles, free]
    x_d = nc.dram_tensor("x_sh", [128, KT, R], F32, kind="ExternalInput").ap()
    W_d = nc.dram_tensor("W_sh", [NHEADS, 128, KT, NHID], F16,
                         kind="ExternalInput").ap()
    B_d = nc.dram_tensor("B_sh", [128, KT, 2 * NHEADS], F32,
                         kind="ExternalInput").ap()
    adj_d = nc.dram_tensor("adj_sh", [128, JT, R], BF16, kind="ExternalInput").ap()
    wo_d = nc.dram_tensor("Wo_sh", [128, KT, BIT], F32, kind="ExternalInput").ap()
    a1o_d = nc.dram_tensor("a1_out", [BIT], F32, kind="ExternalInput").ap()
    a2o_d = nc.dram_tensor("a2_out", [BIT], F32, kind="ExternalInput").ap()
    out_d = nc.dram_tensor("out_rows", [R, BIT], F32, kind="ExternalOutput").ap()

    # collective bounce buffers; Wh per head so each head's allgather overlaps
    # the next head's phase-1 compute. Layout: [128, i*NHID] per rank so the
    # gathered output reads back with 8KB/partition lines.
    ag1_in = [nc.dram_tensor(f"ag1_in{h}", [128, IT * NHID], F16).ap()
              for h in range(NHEADS)]
    ag1_out = [nc.dram_tensor(f"ag1_out{h}", [NC * 128, IT * NHID], F16,
                              addr_space="Shared").ap() for h in range(NHEADS)]
    f_in = nc.dram_tensor("f_in", [1, NHEADS * R], F32).ap()
    f_out = nc.dram_tensor("f_out", [NC, NHEADS * R], F32, addr_space="Shared").ap()
    ag2_in = nc.dram_tensor("ag2_in", [128, IT * (BIT + 1)], F32).ap()
    ag2_out = nc.dram_tensor("ag2_out", [NC * 128, IT * (BIT + 1)], F32,
                             addr_space="Shared").ap()

    rg = [list(range(NC))]

    with tile.TileContext(nc) as tc, ExitStack() as ctx:
        cp = ctx.enter_context(tc.tile_pool(name="const", bufs=1))
        ident = cp.tile([128, 128], F32)
        make_identity(nc, ident)
        ones_col = cp.tile([128, 1], F32)
        nc.vector.memset(ones_col, 1.0)
        ones_row = cp.tile([1, 128], F32)
        nc.vector.memset(ones_row, 1.0)
        a1o_col = cp.tile([BIT, 1], F32)
        nc.sync.dma_start(a1o_col, a1o_d.rearrange("(b one) -> b one", one=1))
        a2o_b = cp.tile([128, BIT], F32)
        nc.sync.dma_start(
            a2o_b, a2o_d.rearrange("(one b) -> one b", one=1).to_broadcast([128, BIT]))
        # adjacency mask, resident for both attention layers (one big DMA)
        adjT = cp.tile([128, JT, R], BF16)
        nc.sync.dma_start(adjT, adj_d)
        # f1 broadcast tiles [128, R] per head
        f1b = [cp.tile([128, R], F32, name=f"f1b_{h}") for h in range(NHEADS)]

        # =============== phase 0: f-logit halves via x @ B ===============
        with tc.tile_pool(name="p0", bufs=1) as p0:
            with tc.tile_pool(name="p0ps", bufs=1, space="PSUM") as p0ps:
                bres = p0.tile([128, KT, 2 * NHEADS], F32R)
                nc.sync.dma_start(bres, B_d.bitcast(F32R))
                xres = p0.tile([128, KT, R], F32R)
                for q in range(4):
                    nc.sync.dma_start(xres[:, q * 8:(q + 1) * 8, :],
                                      x_d[:, q * 8:(q + 1) * 8, :].bitcast(F32R))
                bps = p0ps.tile([2 * NHEADS, R], F32)
                for k in range(KT):
                    nc.tensor.matmul(bps, lhsT=bres[:, k, :], rhs=xres[:, k, :],
                                     start=(k == 0), stop=(k == KT - 1))
                # f2 of my rows, all heads -> allgather (concat on partitions)
                fall = p0.tile([2 * NHEADS, R], F32)
                nc.vector.tensor_copy(fall, bps)
                nc.sync.dma_start(
                    f_in.rearrange("one (h j) -> (one h) j", h=NHEADS),
                    fall[NHEADS:, :])
                nc.gpsimd.collective_compute(
                    "AllGather", OP.bypass, ins=[f_in.opt()], outs=[f_out.opt()],
                    replica_groups=rg)
                for h in range(NHEADS):
                    # row h -> partition 0 (SBUF->SBUF DMA), then broadcast
                    f1row = p0.tile([1, R], F32, name=f"f1row{h}", tag="f1row")
                    nc.sync.dma_start(f1row, fall[h:h + 1, :])
                    fb_ps = p0ps.tile([128, R], F32, name=f"fb_ps{h}", tag="fbps")
                    nc.tensor.matmul(fb_ps, lhsT=ones_row, rhs=f1row,
                                     start=True, stop=True)
                    nc.vector.tensor_copy(f1b[h], fb_ps)

            # =============== phase 1: Wh = x @ W[h] ===============
            with tc.tile_pool(name="p1s", bufs=3) as p1s, \
                 tc.tile_pool(name="p1ps", bufs=1, space="PSUM") as p1ps, \
                 tc.tile_pool(name="p1d", bufs=3) as p1d:
                xp1 = p0.tile([128, KT, R], F16)
                for q in range(4):
                    nc.vector.tensor_copy(xp1[:, q * 8:(q + 1) * 8, :],
                                          xres[:, q * 8:(q + 1) * 8, :].bitcast(F32))
                for h in range(NHEADS):
                    ps = [[p1ps.tile([128, 512], F32, name=f"ps_{h}_{i}_{oh}",
                                     tag=f"ps{i}{oh}") for oh in range(2)]
                          for i in range(IT)]
                    for kb in range(4):
                        wres = p1s.tile([128, 8, NHID], F16, tag="wres")
                        nc.sync.dma_start(
                            wres, W_d[h, :, kb * 8:(kb + 1) * 8, :])
                        for kk in range(8):
                            k = kb * 8 + kk
                            for i in range(IT):
                                for oh in range(2):
                                    nc.tensor.matmul(
                                        ps[i][oh],
                                        lhsT=xp1[:, k, i * 128:(i + 1) * 128],
                                        rhs=wres[:, kk, oh * 512:(oh + 1) * 512],
                                        start=(k == 0), stop=(k == KT - 1),
                                    )
                    for i in range(IT):
                        wh_sb = p1d.tile([128, NHID], F16, tag="wh_sb")
                        nc.vector.tensor_copy(wh_sb[:, :512], ps[i][0])
                        nc.vector.tensor_copy(wh_sb[:, 512:], ps[i][1])
                        nc.sync.dma_start(
                            ag1_in[h][:, i * NHID:(i + 1) * NHID], wh_sb)
                    # allgather this head's Wh while later heads compute
                    nc.gpsimd.collective_compute(
                        "AllGather", OP.bypass, ins=[ag1_in[h].opt()],
                        outs=[ag1_out[h].opt()], replica_groups=rg)

        # =============== phase 2: attention + aggregate, per head ===============
        p2c = ctx.enter_context(tc.tile_pool(name="p2c", bufs=1))
        xcatT = p2c.tile([128, KT, R], F16)
        wof = p2c.tile([128, KT, BIT], F32)
        nc.sync.dma_start(wof, wo_d)
        wob = p2c.tile([128, KT, BIT], F16)
        nc.vector.tensor_copy(wob, wof)

        pps = ctx.enter_context(tc.tile_pool(name="pps", bufs=1, space="PSUM"))
        p2s = ctx.enter_context(tc.tile_pool(name="p2s", bufs=2))
        p2w = ctx.enter_context(tc.tile_pool(name="p2w", bufs=2))
        p2p = ctx.enter_context(tc.tile_pool(name="p2p", bufs=4))

        for h in range(NHEADS):
            # f2 biases for this head: [128, 4, 8] -> (p, i, c)
            f2a = p2s.tile([128, IT, NC], F32, tag="f2a")
            for c in range(NC):
                nc.sync.dma_start(
                    f2a[:, :, c],
                    f_out[c:c + 1, h * R:(h + 1) * R].rearrange(
                        "one (b p) -> (one p) b", p=128))
            b1 = p2s.tile([128, IT, NC], F32, tag="b1")
            nc.vector.tensor_scalar_add(b1, f2a, BIAS_LN)
            b2 = p2s.tile([128, IT, NC], F32, tag="b2")
            nc.vector.tensor_scalar(b2, f2a, ALPHA, BIAS_LN, OP.mult, OP.add)

            rs_acc = p2s.tile([128, R], F32, tag="rs_acc")
            nc.vector.memset(rs_acc, 0.0)
            rs_accB = p2s.tile([128, R], F32, tag="rs_accB")
            nc.vector.memset(rs_accB, 0.0)

            hps = [pps.tile([128, R], F32, name=f"hps{h}_{os}", tag=f"h{os}")
                   for os in range(8)]
            for c in range(NC):
                wht4 = p2w.tile([128, IT, NHID], F16, tag="wht", bufs=3)
                nc.sync.dma_start(
                    wht4, ag1_out[h][c * 128:(c + 1) * 128, :].rearrange(
                        "p (i o) -> p i o", i=IT))
                if h > 0:
                    _elu_tail(c)
                for i in range(IT):
                    jt = c * IT + i
                    e1 = p2p.tile([128, R], BF16, tag="e1")
                    nc.scalar.activation(e1, f1b[h], AF.Exp,
                                         bias=b1[:, i, c:c + 1], scale=1.0)
                    e2 = p2p.tile([128, R], BF16, tag="e2")
                    nc.scalar.activation(e2, f1b[h], AF.Exp,
                                         bias=b2[:, i, c:c + 1], scale=ALPHA)
                    nc.vector.tensor_tensor(e1, e1, e2, OP.max)
                    u = p2p.tile([128, R], BF16, tag="u")
                    nc.vector.tensor_tensor(u, e1, adjT[:, jt, :], OP.mult)
                    if jt % 2 == 0:
                        nc.gpsimd.tensor_tensor(rs_acc, rs_acc, u, OP.add)
                    else:
                        nc.vector.tensor_tensor(rs_accB, rs_accB, u, OP.add)
                    for os in range(8):
                        nc.tensor.matmul(
                            hps[os], lhsT=wht4[:, i, os * 128:(os + 1) * 128],
                            rhs=u, start=(jt == 0), stop=(jt == JT - 1))

            # plain-drain bank 0 so the rowsum matmul can take its slot
            h0sb = p2s.tile([128, R], F32, tag="h0sb")
            nc.vector.tensor_copy(h0sb, hps[0])
            nc.vector.tensor_tensor(rs_acc, rs_acc, rs_accB, OP.add)
            rs_ps = pps.tile([1, R], F32, name=f"rs_ps{h}", tag="h0")
            nc.tensor.matmul(rs_ps, lhsT=ones_col, rhs=rs_acc, start=True, stop=True)
            recip = p2s.tile([1, R], F32, tag="recip")
            nc.vector.reciprocal(recip, rs_ps)
            bc_ps = pps.tile([128, R], F32, name=f"bc_ps{h}", tag="h0")
            nc.tensor.matmul(bc_ps, lhsT=ones_row, rhs=recip, start=True, stop=True)
            rb = p2s.tile([128, R], F32, tag="rb")
            nc.vector.tensor_copy(rb, bc_ps)

            hstage = p2s.tile([128, 8, R], F16, name=f"hstage{h}", tag="hstage",
                              bufs=1)
            for os in range(8):
                nc.vector.tensor_tensor(hstage[:, os, :],
                                        h0sb if os == 0 else hps[os], rb, OP.mult)


            def _elu_tail(os, h=h, hstage=hstage):
                mn = p2w.tile([128, R], F16, tag="u2f")
                nc.vector.tensor_scalar_min(mn, hstage[:, os, :], 0.0)
                ex = p2w.tile([128, R], F16, tag="ex")
                nc.scalar.activation(ex, mn, AF.Exp)
                nc.vector.scalar_tensor_tensor(
                    out=xcatT[:, h * 8 + os, :], in0=ex, scalar=-1.0,
                    in1=hstage[:, os, :], op0=OP.add, op1=OP.max)

        for os in range(8):
            _elu_tail(os)

        # =============== phase 3: Wh2 = x_cat @ W_out; g1/g2 ===============
        wh2T_ps = pps.tile([BIT, R], F32, tag="h2")
        for k in range(KT):
            nc.tensor.matmul(wh2T_ps, lhsT=wob[:, k, :], rhs=xcatT[:, k, :],
                             start=(k == 0), stop=(k == KT - 1))
        wh2T = p2c.tile([BIT, R], F32)
        nc.vector.tensor_copy(wh2T, wh2T_ps)
        g1T_ps = pps.tile([1, R], F32, tag="h3")
        nc.tensor.matmul(g1T_ps, lhsT=a1o_col, rhs=wh2T, start=True, stop=True)
        g1T = p2c.tile([1, R], F32)
        nc.vector.tensor_copy(g1T, g1T_ps)

        for i in range(IT):
            tp_ps = pps.tile([128, BIT], F32, name=f"w2t{i}", tag="h4")
            nc.tensor.transpose(tp_ps, wh2T[:, i * 128:(i + 1) * 128],
                                ident[:BIT, :BIT])
            wh2n = p2w.tile([128, BIT], F32, tag="wh2n")
            nc.vector.tensor_copy(wh2n, tp_ps)
            g2c = p2w.tile([128, 1], F32, tag="g2c")
            scratch2 = p2w.tile([128, BIT], F32, tag="scratch2")
            nc.vector.scalar_tensor_tensor(
                out=scratch2, in0=wh2n, scalar=0.0, in1=a2o_b,
                op0=OP.bypass, op1=OP.mult, accum_out=g2c)
            base = i * (BIT + 1)
            nc.sync.dma_start(ag2_in[:, base:base + BIT], wh2n)
            nc.sync.dma_start(ag2_in[:, base + BIT:base + BIT + 1], g2c)

        nc.gpsimd.collective_compute(
            "AllGather", OP.bypass, ins=[ag2_in.opt()], outs=[ag2_out.opt()],
            replica_groups=rg)

        # =============== phase 4: output attention ===============
        g1b_ps = pps.tile([128, R], F32, tag="h5")
        nc.tensor.matmul(g1b_ps, lhsT=ones_row, rhs=g1T, start=True, stop=True)
        g1b = p2c.tile([128, R], F32)
        nc.vector.tensor_copy(g1b, g1b_ps)

        # 4-way accumulator tree keeps the serial GpSimd chain short
        rs2_acc = [p2s.tile([128, R], F32, name=f"rs2_{a}", tag=f"rs2_{a}")
                   for a in range(4)]
        for a in range(4):
            nc.vector.memset(rs2_acc[a], 0.0)
        ht2_ps = pps.tile([BIT, R], F32, tag="h6")
        for c in range(NC):
            w2t4 = p2w.tile([128, IT, BIT + 1], F32, tag="w2t4")
            nc.sync.dma_start(
                w2t4, ag2_out[c * 128:(c + 1) * 128, :].rearrange(
                    "p (i z) -> p i z", i=IT))
            g2s4 = p2w.tile([128, IT], F32, tag="g2s4")
            nc.vector.tensor_scalar_mul(g2s4, w2t4[:, :, BIT], ALPHA)
            w2b = p2w.tile([128, IT, BIT], F16, tag="w2b")
            nc.vector.tensor_copy(w2b, w2t4[:, :, :BIT])
            for i in range(IT):
                jt = c * IT + i
                e1 = p2p.tile([128, R], BF16, tag="e1")
                nc.scalar.activation(e1, g1b, AF.Exp,
                                     bias=w2t4[:, i, BIT:BIT + 1], scale=1.0)
                e2 = p2p.tile([128, R], BF16, tag="e2")
                nc.scalar.activation(e2, g1b, AF.Exp,
                                     bias=g2s4[:, i:i + 1], scale=ALPHA)
                nc.vector.tensor_tensor(e1, e1, e2, OP.max)
                u2 = p2p.tile([128, R], BF16, tag="u")
                nc.vector.tensor_tensor(u2, e1, adjT[:, jt, :], OP.mult)
                eng = nc.gpsimd if jt % 4 < 2 else nc.vector
                eng.tensor_tensor(rs2_acc[jt % 4], rs2_acc[jt % 4], u2, OP.add)
                nc.tensor.matmul(ht2_ps, lhsT=w2b[:, i, :], rhs=u2,
                                 start=(jt == 0), stop=(jt == JT - 1))

        nc.vector.tensor_tensor(rs2_acc[0], rs2_acc[0], rs2_acc[1], OP.add)
        nc.vector.tensor_tensor(rs2_acc[2], rs2_acc[2], rs2_acc[3], OP.add)
        nc.vector.tensor_tensor(rs2_acc[0], rs2_acc[0], rs2_acc[2], OP.add)
        rs2_ps = pps.tile([1, R], F32, tag="h7")
        nc.tensor.matmul(rs2_ps, lhsT=ones_col, rhs=rs2_acc[0], start=True, stop=True)
        recip2 = p2c.tile([1, R], F32)
        nc.vector.reciprocal(recip2, rs2_ps)
        bc2_ps = pps.tile([128, R], F32, tag="h0")
        nc.tensor.matmul(bc2_ps, lhsT=ones_row, rhs=recip2, start=True, stop=True)
        rb2 = p2c.tile([128, R], F32)
        nc.vector.tensor_copy(rb2, bc2_ps)

        ot = p2c.tile([BIT, R], F32)
        nc.vector.tensor_tensor(ot, ht2_ps, rb2[:BIT, :], OP.mult)
        outT = p2c.tile([BIT, R], F32)
        nc.scalar.activation(outT, ot, AF.Tanh)
        for i in range(IT):
            tp_ps = pps.tile([128, BIT], F32, name=f"ot{i}", tag="h1")
            nc.tensor.transpose(tp_ps, outT[:, i * 128:(i + 1) * 128],
                                ident[:BIT, :BIT])
            ob = p2w.tile([128, BIT], F32, tag="ob")
            nc.vector.tensor_copy(ob, tp_ps)
            nc.sync.dma_start(out_d[i * 128:(i + 1) * 128, :], ob)

    _split_excess_waits(nc, max_waits=1)
    return nc


_CACHED = None


def _get_program():
    global _CACHED
    if _CACHED is None:
        _CACHED = build_program()
    return _CACHED


def _interleave(a, kt):
    """[kt*128, free...] -> [128, kt, free...] partition-major."""
    return np.ascontiguousarray(
        a.reshape(kt, 128, *a.shape[1:]).transpose(1, 0, *range(2, a.ndim + 1)))


def make_in_maps(x, adj, W, a1, a2, W_out, a1_out, a2_out):
    import ml_dtypes
    xT = np.ascontiguousarray(x.T)
    adjT_bf = adj.T.astype(ml_dtypes.bfloat16)
    # B = [W[h] @ a1[h] (4 cols) | W[h] @ a2[h] (4 cols)]  (fp32 logit vecs)
    B = np.concatenate(
        [np.stack([W[h] @ a1[h] for h in range(NHEADS)], axis=1),
         np.stack([W[h] @ a2[h] for h in range(NHEADS)], axis=1)],
        axis=1).astype(np.float32)
    # W interleaved: [h, 128, KT, NHID]
    W_sh = np.ascontiguousarray(
        W.reshape(NHEADS, KT, 128, NHID).transpose(0, 2, 1, 3)
).astype(np.float16)
    B_sh = _interleave(B, KT)
    Wo_sh = _interleave(W_out, KT)
    in_maps = []
    for d in range(NC):
        cols = slice(d * R, (d + 1) * R)
        in_maps.append({
            "x_sh": _interleave(np.ascontiguousarray(xT[:, cols]), KT),
            "W_sh": W_sh,
            "B_sh": B_sh,
            "adj_sh": _interleave(np.ascontiguousarray(adjT_bf[:, cols]), JT),
            "Wo_sh": Wo_sh,
            "a1_out": a1_out, "a2_out": a2_out,
        })
    return in_maps


def kernel(x, adj, W, a1, a2, W_out, a1_out, a2_out, _trace=False):
    nc = _get_program()
    in_maps = make_in_maps(np.asarray(x, np.float32), np.asarray(adj, np.float32),
                           np.asarray(W, np.float32), np.asarray(a1, np.float32),
                           np.asarray(a2, np.float32), np.asarray(W_out, np.float32),
                           np.asarray(a1_out, np.float32),
                           np.asarray(a2_out, np.float32))
    res = bass_utils.run_bass_kernel_spmd(
        nc, in_maps, core_ids=list(range(NC)), trace=_trace)
    out = np.concatenate([res.results[d]["out_rows"] for d in range(NC)], axis=0)
    if _trace:
        kernel.last_exec_time_ns = res.exec_time_ns
        kernel.last_results = res
    return out


# revision 35
# speedup vs baseline: 1.0446x; 1.0104x over previous
"""Trainium2 Bass kernel for nn_GATNet_IMG (dense 2-layer GAT, N=4096).

Sharding: 1D row-parallel over the node dim across 8 NeuronCores.
Each core computes Wh for its 512 rows (all 4 heads), AllGathers Wh
per head (overlapped with the next head's matmuls), then computes its
[512, 4096] attention block per head with a fused masked softmax (no
NxN matrix ever hits HBM), aggregates h^T = Wh^T @ u on TensorE, and
repeats the same pattern for the output attention layer.

Key tricks:
  - exp(leaky_relu(s)) == max(exp(s), exp(0.2*s))  (exact, all s)
  - attention logit halves f1/f2 come from one tiny [8]-wide matmul
    x @ B where B = [W[h]@a1[h] | W[h]@a2[h]] is host-precomputed, so
    logit precision is independent of the big-GEMM compute dtype
  - unnormalized attention + row-sum via ones-matmul; normalization is
    a post-scale of h^T (free-axis scale via a broadcast matmul)
  - elu(x) == max(exp(min(x, 0)) - 1, x)           (exact)
  - ln(2^-30) folded into the exp bias keeps row sums in DVE
    reciprocal range
  - host-side sharding pre-transposes x/adj AND interleaves all heavy
    streams into [128, ktiles, free] partition-major layouts so every
    DMA moves 8-64KB contiguous per partition (packet-rate limit)
"""

import math
from contextlib import ExitStack

import numpy as np

import concourse.bass as bass
import concourse.mybir as mybir
import concourse.tile as tile
from concourse import bass_utils
from concourse.masks import make_identity

F32 = mybir.dt.float32
F32R = mybir.dt.float32r
F16 = mybir.dt.float16
BF16 = mybir.dt.bfloat16
AF = mybir.ActivationFunctionType
OP = mybir.AluOpType

N = 4096
NFEAT = 4096
NHID = 1024
NHEADS = 4
BIT = 64
NC = 8
R = N // NC          # 512 attention rows per core
KT = NFEAT // 128    # 32 k tiles
JT = N // 128        # 32 node-column tiles
IT = R // 128        # 4 row tiles per core
ALPHA = 0.2
BIAS_LN = -30.0 * math.log(2.0)   # ln(2^-30) folded into exp bias (layer 1)


def _split_excess_waits(nc, max_waits=1):
    """walrus codegen rejects >max_waits sync-wait commands per instruction;
    push excess waits onto preceding same-engine NoOps."""
    n_fixed = 0
    for f in nc.m.functions:
        for b in f.blocks:
            new_insts = []
            changed = False
            for inst in b.instructions:
                si = getattr(inst, "sync_info", None)
                if si is not None and si.on_wait and len(si.on_wait) > max_waits:
                    waits = list(si.on_wait)
                    excess, keep = waits[:-max_waits], waits[-max_waits:]
                    for ci in range(0, len(excess), max_waits):
                        nop = mybir.InstNoOp(
                            name=f"{inst.name}-ws{ci}",
                            sync_info=mybir.SyncInfo(
                                on_wait=excess[ci:ci + max_waits], on_update=[]
                            ),
                            bass_nofuse=True,
                            engine=inst.engine,
                        )
                        new_insts.append(nop)
                    inst.sync_info = mybir.SyncInfo(
                        on_wait=keep, on_update=list(si.on_update or [])
                    )
                    n_fixed += 1
                    changed = True
                new_insts.append(inst)
            if changed:
                insts = b.instructions
                try:
                    b.instructions = new_insts
                except Exception:
                    while len(insts):
                        insts.pop()
                    for i in new_insts:
                        insts.append(i)
    return n_fixed


def build_program():
    nc = bass.Bass("TRN2", target_bir_lowering=False, debug=False, num_devices=NC)

    # host-interleaved inputs: [128 partitions, ktiles, free]
    x_d = nc.dram_tensor("x_sh", [128, KT, R], F32, kind="ExternalInput").ap()
    W_d = nc.dram_tensor("W_sh", [NHEADS, 128, KT, NHID], F16,
                         kind="ExternalInput").ap()
    B_d = nc.dram_tensor("B_sh", [128, KT, 2 * NHEADS], F32,
                         kind="ExternalInput").ap()
    adj_d = nc.dram_tensor("adj_sh", [128, JT, R], BF16, kind="ExternalInput").ap()
    wo_d = nc.dram_tensor("Wo_sh", [128, KT, BIT], F32, kind="ExternalInput").ap()
    a1o_d = nc.dram_tensor("a1_out", [BIT], F32, kind="ExternalInput").ap()
    a2o_d = nc.dram_tensor("a2_out", [BIT], F32, kind="ExternalInput").ap()
    out_d = nc.dram_tensor("out_rows", [R, BIT], F32, kind="ExternalOutput").ap()

    # collective bounce buffers; Wh per head so each head's allgather overlaps
    # the next head's phase-1 compute. Layout: [128, i*NHID] per rank so the
    # gathered output reads back with 8KB/partition lines.
    ag1_in = [nc.dram_tensor(f"ag1_in{h}", [128, IT * NHID], F16).ap()
              for h in range(NHEADS)]
    ag1_out = [nc.dram_tensor(f"ag1_out{h}", [NC * 128, IT * NHID], F16,
                              addr_space="Shared").ap() for h in range(NHEADS)]
    f_in = nc.dram_tensor("f_in", [1, NHEADS * R], F32).ap()
    f_out = nc.dram_tensor("f_out", [NC, NHEADS * R], F32, addr_space="Shared").ap()
    ag2_in = nc.dram_tensor("ag2_in", [128, IT * (BIT + 1)], F32).ap()
    ag2_out = nc.dram_tensor("ag2_out", [NC * 128, IT * (BIT + 1)], F32,
                             addr_space="Shared").ap()

    rg = [list(range(NC))]

    with tile.TileContext(nc) as tc, ExitStack() as ctx:
        cp = ctx.enter_context(tc.tile_pool(name="const", bufs=1))
        ident = cp.tile([128, 128], F32)
        make_identity(nc, ident)
        ones_col = cp.tile([128, 1], F32)
        nc.vector.memset(ones_col, 1.0)
        ones_row = cp.tile([1, 128], F32)
        nc.vector.memset(ones_row, 1.0)
        a1o_col = cp.tile([BIT, 1], F32)
        nc.sync.dma_start(a1o_col, a1o_d.rearrange("(b one) -> b one", one=1))
        a2o_b = cp.tile([128, BIT], F32)
        nc.sync.dma_start(
            a2o_b, a2o_d.rearrange("(one b) -> one b", one=1).to_broadcast([128, BIT]))
        # adjacency mask, resident for both attention layers (one big DMA)
        adjT = cp.tile([128, JT, R], BF16)
        nc.sync.dma_start(adjT, adj_d)
        # f1 broadcast tiles [128, R] per head
        f1b = [cp.tile([128, R], F32, name=f"f1b_{h}") for h in range(NHEADS)]

        # =============== phase 0: f-logit halves via x @ B ===============
        with tc.tile_pool(name="p0", bufs=1) as p0:
            with tc.tile_pool(name="p0ps", bufs=1, space="PSUM") as p0ps:
                bres = p0.tile([128, KT, 2 * NHEADS], F32R)
                nc.sync.dma_start(bres, B_d.bitcast(F32R))
                xres = p0.tile([128, KT, R], F32R)
                for q in range(4):
                    nc.sync.dma_start(xres[:, q * 8:(q + 1) * 8, :],
                                      x_d[:, q * 8:(q + 1) * 8, :].bitcast(F32R))
                bps = p0ps.tile([2 * NHEADS, R], F32)
                for k in range(KT):
                    nc.tensor.matmul(bps, lhsT=bres[:, k, :], rhs=xres[:, k, :],
                                     start=(k == 0), stop=(k == KT - 1))
                # f2 of my rows, all heads -> allgather (concat on partitions)
                fall = p0.tile([2 * NHEADS, R], F32)
                nc.vector.tensor_copy(fall, bps)
                nc.sync.dma_start(
                    f_in.rearrange("one (h j) -> (one h) j", h=NHEADS),
                    fall[NHEADS:, :])
                nc.gpsimd.collective_compute(
                    "AllGather", OP.bypass, ins=[f_in.opt()], outs=[f_out.opt()],
                    replica_groups=rg)
                for h in range(NHEADS):
                    # row h -> partition 0 (SBUF->SBUF DMA), then broadcast
                    f1row = p0.tile([1, R], F32, name=f"f1row{h}", tag="f1row")
                    nc.sync.dma_start(f1row, fall[h:h + 1, :])
                    fb_ps = p0ps.tile([128, R], F32, name=f"fb_ps{h}", tag="fbps")
                    nc.tensor.matmul(fb_ps, lhsT=ones_row, rhs=f1row,
                                     start=True, stop=True)
                    nc.vector.tensor_copy(f1b[h], fb_ps)

            # =============== phase 1: Wh = x @ W[h] ===============
            with tc.tile_pool(name="p1s", bufs=3) as p1s, \
                 tc.tile_pool(name="p1ps", bufs=1, space="PSUM") as p1ps, \
                 tc.tile_pool(name="p1d", bufs=3) as p1d:
                xp1 = p0.tile([128, KT, R], F16)
                for q in range(4):
                    nc.vector.tensor_copy(xp1[:, q * 8:(q + 1) * 8, :],
                                          xres[:, q * 8:(q + 1) * 8, :].bitcast(F32))
                for h in range(NHEADS):
                    ps = [[p1ps.tile([128, 512], F32, name=f"ps_{h}_{i}_{oh}",
                                     tag=f"ps{i}{oh}") for oh in range(2)]
                          for i in range(IT)]
                    for kb in range(4):
                        wres = p1s.tile([128, 8, NHID], F16, tag="wres")
                        nc.sync.dma_start(
                            wres, W_d[h, :, kb * 8:(kb + 1) * 8, :])
                        for kk in range(8):
                            k = kb * 8 + kk
                            for i in range(IT):
                                for oh in range(2):
                                    nc.tensor.matmul(
                                        ps[i][oh],
                                        lhsT=xp1[:, k, i * 128:(i + 1) * 128],
                                        rhs=wres[:, kk, oh * 512:(oh + 1) * 512],
                                        start=(k == 0), stop=(k == KT - 1),
                                    )
                    for i in range(IT):
                        wh_sb = p1d.tile([128, NHID], F16, tag="wh_sb")
                        nc.vector.tensor_copy(wh_sb[:, :512], ps[i][0])
                        nc.scalar.copy(wh_sb[:, 512:], ps[i][1])
                        nc.sync.dma_start(
                            ag1_in[h][:, i * NHID:(i + 1) * NHID], wh_sb)
                    # allgather this head's Wh while later heads compute
                    nc.gpsimd.collective_compute(
                        "AllGather", OP.bypass, ins=[ag1_in[h].opt()],
                        outs=[ag1_out[h].opt()], replica_groups=rg)

        # =============== phase 2: attention + aggregate, per head ===============
        p2c = ctx.enter_context(tc.tile_pool(name="p2c", bufs=1))
        xcatT = p2c.tile([128, KT, R], F16)
        wof = p2c.tile([128, KT, BIT], F32)
        nc.sync.dma_start(wof, wo_d)
        wob = p2c.tile([128, KT, BIT], F16)
        nc.vector.tensor_copy(wob, wof)

        pps = ctx.enter_context(tc.tile_pool(name="pps", bufs=1, space="PSUM"))
        p2s = ctx.enter_context(tc.tile_pool(name="p2s", bufs=2))
        p2w = ctx.enter_context(tc.tile_pool(name="p2w", bufs=2))
        p2p = ctx.enter_context(tc.tile_pool(name="p2p", bufs=4))

        for h in range(NHEADS):
            # f2 biases for this head: [128, 4, 8] -> (p, i, c)
            f2a = p2s.tile([128, IT, NC], F32, tag="f2a")
            for c in range(NC):
                nc.sync.dma_start(
                    f2a[:, :, c],
                    f_out[c:c + 1, h * R:(h + 1) * R].rearrange(
                        "one (b p) -> (one p) b", p=128))
            b1 = p2s.tile([128, IT, NC], F32, tag="b1")
            nc.vector.tensor_scalar_add(b1, f2a, BIAS_LN)
            b2 = p2s.tile([128, IT, NC], F32, tag="b2")
            nc.vector.tensor_scalar(b2, f2a, ALPHA, BIAS_LN, OP.mult, OP.add)

            rs_acc = p2s.tile([128, R], F32, tag="rs_acc")
            nc.vector.memset(rs_acc, 0.0)
            rs_accB = p2s.tile([128, R], F32, tag="rs_accB")
            nc.vector.memset(rs_accB, 0.0)

            hps = [pps.tile([128, R], F32, name=f"hps{h}_{os}", tag=f"h{os}")
                   for os in range(8)]
            for c in range(NC):
                wht4 = p2w.tile([128, IT, NHID], F16, tag="wht", bufs=3)
                nc.sync.dma_start(
                    wht4, ag1_out[h][c * 128:(c + 1) * 128, :].rearrange(
                        "p (i o) -> p i o", i=IT))
                if h > 0:
                    _elu_tail(c)
                for i in range(IT):
                    jt = c * IT + i
                    e1 = p2p.tile([128, R], BF16, tag="e1")
                    nc.scalar.activation(e1, f1b[h], AF.Exp,
                                         bias=b1[:, i, c:c + 1], scale=1.0)
                    e2 = p2p.tile([128, R], BF16, tag="e2")
                    nc.scalar.activation(e2, f1b[h], AF.Exp,
                                         bias=b2[:, i, c:c + 1], scale=ALPHA)
                    nc.vector.tensor_tensor(e1, e1, e2, OP.max)
                    u = p2p.tile([128, R], BF16, tag="u")
                    nc.vector.tensor_tensor(u, e1, adjT[:, jt, :], OP.mult)
                    if jt % 2 == 0:
                        nc.gpsimd.tensor_tensor(rs_acc, rs_acc, u, OP.add)
                    else:
                        nc.vector.tensor_tensor(rs_accB, rs_accB, u, OP.add)
                    for os in range(8):
                        nc.tensor.matmul(
                            hps[os], lhsT=wht4[:, i, os * 128:(os + 1) * 128],
                            rhs=u, start=(jt == 0), stop=(jt == JT - 1))

            # plain-drain bank 0 so the rowsum matmul can take its slot
            h0sb = p2s.tile([128, R], F32, tag="h0sb")
            nc.vector.tensor_copy(h0sb, hps[0])
            nc.vector.tensor_tensor(rs_acc, rs_acc, rs_accB, OP.add)
            rs_ps = pps.tile([1, R], F32, name=f"rs_ps{h}", tag="h0")
            nc.tensor.matmul(rs_ps, lhsT=ones_col, rhs=rs_acc, start=True, stop=True)
            recip = p2s.tile([1, R], F32, tag="recip")
            nc.vector.reciprocal(recip, rs_ps)
            bc_ps = pps.tile([128, R], F32, name=f"bc_ps{h}", tag="h0")
            nc.tensor.matmul(bc_ps, lhsT=ones_row, rhs=recip, start=True, stop=True)
            rb = p2s.tile([128, R], F32, tag="rb")
            nc.vector.tensor_copy(rb, bc_ps)

            hstage = p2s.tile([128, 8, R], F16, name=f"hstage{h}", tag="hstage",
                              bufs=1)
            for os in range(8):
                nc.vector.tensor_tensor(hstage[:, os, :],
                                        h0sb if os == 0 else hps[os], rb, OP.mult)


            def _elu_tail(os, h=h, hstage=hstage):
                mn = p2w.tile([128, R], F16, tag="u2f")
                nc.vector.tensor_scalar_min(mn, hstage[:, os, :], 0.0)
                ex = p2w.tile([128, R], F16, tag="ex")
                nc.scalar.activation(ex, mn, AF.Exp)
                nc.vector.scalar_tensor_tensor(
                    out=xcatT[:, h * 8 + os, :], in0=ex, scalar=-1.0,
                    in1=hstage[:, os, :], op0=OP.add, op1=OP.max)

        for os in range(8):
            _elu_tail(os)

        # =============== phase 3: Wh2 = x_cat @ W_out; g1/g2 ===============
        wh2T_ps = pps.tile([BIT, R], F32, tag="h2")
        for k in range(KT):
            nc.tensor.matmul(wh2T_ps, lhsT=wob[:, k, :], rhs=xcatT[:, k, :],
                             start=(k == 0), stop=(k == KT - 1))
        wh2T = p2c.tile([BIT, R], F32)
        nc.vector.tensor_copy(wh2T, wh2T_ps)
        g1T_ps = pps.tile([1, R], F32, tag="h3")
        nc.tensor.matmul(g1T_ps, lhsT=a1o_col, rhs=wh2T, start=True, stop=True)
        g1T = p2c.tile([1, R], F32)
        nc.vector.tensor_copy(g1T, g1T_ps)

        for i in range(IT):
            tp_ps = pps.tile([128, BIT], F32, name=f"w2t{i}", tag="h4")
            nc.tensor.transpose(tp_ps, wh2T[:, i * 128:(i + 1) * 128],
                                ident[:BIT, :BIT])
            wh2n = p2w.tile([128, BIT], F32, tag="wh2n")
            nc.vector.tensor_copy(wh2n, tp_ps)
            g2c = p2w.tile([128, 1], F32, tag="g2c")
            scratch2 = p2w.tile([128, BIT], F32, tag="scratch2")
            nc.vector.scalar_tensor_tensor(
                out=scratch2, in0=wh2n, scalar=0.0, in1=a2o_b,
                op0=OP.bypass, op1=OP.mult, accum_out=g2c)
            base = i * (BIT + 1)
            nc.sync.dma_start(ag2_in[:, base:base + BIT], wh2n)
            nc.sync.dma_start(ag2_in[:, base + BIT:base + BIT + 1], g2c)

        nc.gpsimd.collective_compute(
            "AllGather", OP.bypass, ins=[ag2_in.opt()], outs=[ag2_out.opt()],
            replica_groups=rg)

        # =============== phase 4: output attention ===============
        g1b_ps = pps.tile([128, R], F32, tag="h5")
        nc.tensor.matmul(g1b_ps, lhsT=ones_row, rhs=g1T, start=True, stop=True)
        g1b = p2c.tile([128, R], F32)
        nc.vector.tensor_copy(g1b, g1b_ps)

        # 4-way accumulator tree keeps the serial GpSimd chain short
        rs2_acc = [p2s.tile([128, R], F32, name=f"rs2_{a}", tag=f"rs2_{a}")
                   for a in range(4)]
        for a in range(4):
            nc.vector.memset(rs2_acc[a], 0.0)
        ht2_ps = pps.tile([BIT, R], F32, tag="h6")
        for c in range(NC):
            w2t4 = p2w.tile([128, IT, BIT + 1], F32, tag="w2t4")
            nc.sync.dma_start(
                w2t4, ag2_out[c * 128:(c + 1) * 128, :].rearrange(
                    "p (i z) -> p i z", i=IT))
            g2s4 = p2w.tile([128, IT], F32, tag="g2s4")
            nc.vector.tensor_scalar_mul(g2s4, w2t4[:, :, BIT], ALPHA)
            w2b = p2w.tile([128, IT, BIT], F16, tag="w2b")
            nc.vector.tensor_copy(w2b, w2t4[:, :, :BIT])
            for i in range(IT):
                jt = c * IT + i
                e1 = p2p.tile([128, R], BF16, tag="e1")
                nc.scalar.activation(e1, g1b, AF.Exp,
                                     bias=w2t4[:, i, BIT:BIT + 1], scale=1.0)
                e2 = p2p.tile([128, R], BF16, tag="e2")
                nc.scalar.activation(e2, g1b, AF.Exp,
                                     bias=g2s4[:, i:i + 1], scale=ALPHA)
                nc.vector.tensor_tensor(e1, e1, e2, OP.max)
                u2 = p2p.tile([128, R], BF16, tag="u")
                nc.vector.tensor_tensor(u2, e1, adjT[:, jt, :], OP.mult)
                eng = nc.gpsimd if jt % 4 < 2 else nc.vector
                eng.tensor_tensor(rs2_acc[jt % 4], rs2_acc[jt % 4], u2, OP.add)
                nc.tensor.matmul(ht2_ps, lhsT=w2b[:, i, :], rhs=u2,
                                 start=(jt == 0), stop=(jt == JT - 1))

        nc.vector.tensor_tensor(rs2_acc[0], rs2_acc[0], rs2_acc[1], OP.add)
        nc.vector.tensor_tensor(rs2_acc[2], rs2_acc[2], rs2_acc[3], OP.add)
        nc.vector.tensor_tensor(rs2_acc[0], rs2_acc[0], rs2_acc[2], OP.add)
        rs2_ps = pps.tile([1, R], F32, tag="h7")
        nc.tensor.matmul(rs2_ps, lhsT=ones_col, rhs=rs2_acc[0], start=True, stop=True)
        recip2 = p2c.tile([1, R], F32)
        nc.vector.reciprocal(recip2, rs2_ps)
        bc2_ps = pps.tile([128, R], F32, tag="h0")
        nc.tensor.matmul(bc2_ps, lhsT=ones_row, rhs=recip2, start=True, stop=True)
        rb2 = p2c.tile([128, R], F32)
        nc.vector.tensor_copy(rb2, bc2_ps)

        ot = p2c.tile([BIT, R], F32)
        nc.vector.tensor_tensor(ot, ht2_ps, rb2[:BIT, :], OP.mult)
        outT = p2c.tile([BIT, R], F32)
        nc.scalar.activation(outT, ot, AF.Tanh)
        for i in range(IT):
            tp_ps = pps.tile([128, BIT], F32, name=f"ot{i}", tag="h1")
            nc.tensor.transpose(tp_ps, outT[:, i * 128:(i + 1) * 128],
                                ident[:BIT, :BIT])
            ob = p2w.tile([128, BIT], F32, tag="ob")
            nc.vector.tensor_copy(ob, tp_ps)
            nc.sync.dma_start(out_d[i * 128:(i + 1) * 128, :], ob)

    _split_excess_waits(nc, max_waits=1)
    return nc


_CACHED = None


def _get_program():
    global _CACHED
    if _CACHED is None:
        _CACHED = build_program()
    return _CACHED


def _interleave(a, kt):
    """[kt*128, free...] -> [128, kt, free...] partition-major."""
    return np.ascontiguousarray(
        a.reshape(kt, 128, *a.shape[1:]).transpose(1, 0, *range(2, a.ndim + 1)))


def make_in_maps(x, adj, W, a1, a2, W_out, a1_out, a2_out):
    import ml_dtypes
    xT = np.ascontiguousarray(x.T)
    adjT_bf = adj.T.astype(ml_dtypes.bfloat16)
    # B = [W[h] @ a1[h] (4 cols) | W[h] @ a2[h] (4 cols)]  (fp32 logit vecs)
    B = np.concatenate(
        [np.stack([W[h] @ a1[h] for h in range(NHEADS)], axis=1),
         np.stack([W[h] @ a2[h] for h in range(NHEADS)], axis=1)],
        axis=1).astype(np.float32)
    # W interleaved: [h, 128, KT, NHID]
    W_sh = np.ascontiguousarray(
        W.reshape(NHEADS, KT, 128, NHID).transpose(0, 2, 1, 3)
).astype(np.float16)
    B_sh = _interleave(B, KT)
    Wo_sh = _interleave(W_out, KT)
    in_maps = []
    for d in range(NC):
        cols = slice(d * R, (d + 1) * R)
        in_maps.append({
            "x_sh": _interleave(np.ascontiguousarray(xT[:, cols]), KT),
            "W_sh": W_sh,
            "B_sh": B_sh,
            "adj_sh": _interleave(np.ascontiguousarray(adjT_bf[:, cols]), JT),
            "Wo_sh": Wo_sh,
            "a1_out": a1_out, "a2_out": a2_out,
        })
    return in_maps


def kernel(x, adj, W, a1, a2, W_out, a1_out, a2_out, _trace=False):
    nc = _get_program()
    in_maps = make_in_maps(np.asarray(x, np.float32), np.asarray(adj, np.float32),
                           np.asarray(W, np.float32), np.asarray(a1, np.float32),
                           np.asarray(a2, np.float32), np.asarray(W_out, np.float32),
                           np.asarray(a1_out, np.float32),
                           np.asarray(a2_out, np.float32))
    res = bass_utils.run_bass_kernel_spmd(
        nc, in_maps, core_ids=list(range(NC)), trace=_trace)
    out = np.concatenate([res.results[d]["out_rows"] for d in range(NC)], axis=0)
    if _trace:
        kernel.last_exec_time_ns = res.exec_time_ns
        kernel.last_results = res
    return out
